# revision 64
# baseline (speedup 1.0000x reference)
"""BiMamba block on 8 Trainium2 NeuronCores via Bass/Tile.

Sharding (SPMD, one shared NEFF, no collectives):
  core c: dir = c//4 (0=fwd, 1=bwd), batch = (c//2)%2, half = c%2.
Each core runs the full mamba pipeline for one (dir, batch) pair on its
half of d_inner (scan channels are independent), computing the full-d_inner
xi/conv/x_proj path locally (dt/B/C need the full d_inner contraction).
The d_inner axis is permuted per core so its own half is always blocks 0..7,
keeping the program identical across cores. Each core emits a partial
output (d_model, L) = (y_half @ out_w_half) @ proj_w_dir, transposed;
the host sums the 8 partials, un-reverses the bwd direction, adds proj_b.

Layouts: everything on-chip is "transposed" (feature dim on partitions,
time on the free axis) so the causal conv is a free-dim shift, the scan
runs along the free axis (DVE tensor_tensor_scan), and every matmul uses
naturally-laid-out weights as the stationary lhsT operand.

v2 engine split (the DVE scan is the hard floor: 2 cyc/elem dependency-
bound, 17.2us per state, 16 states back-to-back = ~437us of the runtime):
  PE:  in_proj xi, x_proj, dt, selector/diag(D) broadcasts, z, identity
       y-accumulate, out_proj.
  DVE: causal conv (per-tap scalar_tensor_tensor), u/b/C mults, scan, y-add.
  ACT: xi PSUM evacuation, silu(+bias), softplus (Exp+Ln), dA exps, fused
       z silu-evacuation.
Per state n: dA = exp(A[:,n]*dt) (ACT, double-buffered one state ahead),
b = u*B_n (DVE, partition-broadcast B), h = scan(dA, b) with chain-reset
via GpSimd dA=0 memsets at block boundaries (a DVE memset would park the
in-order DVE queue on the exp), y += h*C_n.

y accumulation: blocks 0-3 DVE-add in SBUF fp16; blocks 4-6 + 7-lo via PE
identity matmuls into 7 PSUM banks; block 7-hi DVE-adds into y2's dead
second half. The 8th PSUM bank runs the z matmuls DURING the early scan
states. Pool-overlap deps are pool-granular (an alloc over a released zone
waits for the WHOLE old pool), so x^T + streamed w_z chunks live in their
own never-overlapped pool (pha_late) and the rest of phase A (pha) is
released right after u/seed, letting scan 0 start at ~129us.

B/C rows: states 0/1 are broadcast by PE one-hot-row selector matmuls
(the DMA broadcast has ~25us latency); later pairs are DMA-broadcast from
a DRAM staging buffer, prefetched one pair ahead on both HWDGE queues.
State 15 is split into two half scans and the output partial is fp16 so
the tail (gate + fused out_w@proj_w matmul, 2-bank PSUM groups, streamed
weights) finishes ~56us after the last scan starts.
"""

import numpy as np

B, L, D = 2, 1024, 1024
DI, DH, NST, RNK = 2048, 1024, 16, 64
NBLK = DH // 128          # 8 d-blocks per half
NBLK_F = DI // 128        # 16 d-blocks full
F16 = np.float16

_CACHE = {}


def _build_module(sim_compat=False, a_imm=None):
    """sim_compat=True replaces Silu (absent from CoreSim) with
    Sigmoid + multiply; the hardware build uses the Silu table directly."""
    import concourse.bass as bass
    import concourse.mybir as mybir
    from concourse import bacc
    from concourse.tile import TileContext

    dt = mybir.dt
    AF = mybir.ActivationFunctionType
    OP = mybir.AluOpType

    nc = bacc.Bacc("TRN2", target_bir_lowering=False, debug=False)

    # ---- DRAM I/O ----
    xT_d = nc.dram_tensor("xT", (D, L), dt.float16, kind="ExternalInput")
    w_xi_d = nc.dram_tensor("w_xi", (NBLK_F, 128, 8, 128), dt.float16, kind="ExternalInput")
    w_z_d = nc.dram_tensor("w_z", (8, 128, 8, 128), dt.float16, kind="ExternalInput")
    conv_w_d = nc.dram_tensor("conv_w", (128, NBLK_F, 4), dt.float32, kind="ExternalInput")
    conv_b_d = nc.dram_tensor("conv_b", (DI,), dt.float32, kind="ExternalInput")
    xp_w_d = nc.dram_tensor("xp_w", (DI, 128), dt.float16, kind="ExternalInput")
    dt_w_d = nc.dram_tensor("dt_w", (RNK, DH), dt.float16, kind="ExternalInput")
    dt_b_d = nc.dram_tensor("dt_b", (DH,), dt.float32, kind="ExternalInput")
    A_d = None
    if a_imm is None:
        A_d = nc.dram_tensor("A", (DH, NST), dt.float32, kind="ExternalInput")
    dskip_d = nc.dram_tensor("dskip", (DH,), dt.float32, kind="ExternalInput")
    w2_d = nc.dram_tensor("w2", (8, 128, 8, 128), dt.float16, kind="ExternalInput")
    ident_d = nc.dram_tensor("ident", (128, 128), dt.float16, kind="ExternalInput")
    # one-hot-row selectors: sel[k, j, :] = 1.0 iff k == j (j = state 0/1)
    sel_d = nc.dram_tensor("sel", (NST, 2, 128), dt.float16, kind="ExternalInput")
    # diag(D) per d-block 4..7, the PSUM seed weights for the y skip term
    dd_d = nc.dram_tensor("dskip_diag", (128, 4, 128), dt.float16, kind="ExternalInput")
    pT_d = nc.dram_tensor("pT", (D, L), dt.float16, kind="ExternalOutput")

    with TileContext(nc) as tc:
        psum = tc.alloc_tile_pool(name="psum", bufs=6, space="PSUM")
        const = tc.alloc_tile_pool(name="const", bufs=1)
        persist = tc.alloc_tile_pool(name="persist", bufs=1)
        dram = tc.alloc_tile_pool(name="dram", bufs=1, space="DRAM")
        # B/C rows staged in DRAM so they can be partition-broadcast by DMA
        bc_stage = dram.tile([2 * NST, L], dt.float16)

        # ---- constants read during the scan phase (everything phase-A-only
        # lives in pha_early so its space is reclaimed for the scan pools) ----
        A_sb = None
        if a_imm is None:
            A_sb = const.tile([128, NBLK, NST], dt.float32)
        dskip_sb = const.tile([128, NBLK], dt.float32)
        ident_sb = const.tile([128, 128], dt.float16)
        dd_sb = const.tile([128, 4, 128], dt.float16)

        # ---- persistent activations ----
        zT = persist.tile([128, NBLK, L], dt.float16)
        dtT = persist.tile([128, NBLK, L], dt.float16)
        u2 = persist.tile([128, NBLK * L], dt.float16)
        y2 = persist.tile([128, NBLK * L], dt.float16)
        u3 = u2.rearrange("p (g t) -> p g t", g=NBLK)
        y3 = y2.rearrange("p (g t) -> p g t", g=NBLK)
        # pair-0 B/C broadcasts (PE selector matmuls) alias y2 blocks 4-7,
        # which accumulate in PSUM and never touch their SBUF half; dA(0)
        # needs the persist pool (both are produced while the phase A pools
        # still hold all of SBUF, so they cannot live in phb)
        bc0 = y2[:, 4 * L:8 * L].rearrange("p (q t) -> p q t", q=4)
        dA0_p = persist.tile([128, NBLK * L], dt.float16)
        # block 7 upper-half skip term D*xc, staged until its y accumulator
        # region (the tail of bc0) frees up after state 1
        pad7skip = persist.tile([128, 512], dt.float16)

        # ================= phase A: in_proj, conv, x_proj, dt =================
        # Pool-overlap dependencies are pool-granular: a pool allocated over
        # a released zone waits for the WHOLE old pool to drain. The z
        # matmuls keep reading x^T until deep into the scan phase, so x^T
        # (plus the streamed w_z chunks) lives in its own pool, kept alive
        # until the end; everything else phase-A is in pha_early, released
        # right after dbc/u so the scan pools start immediately.
        pha_late = tc.alloc_tile_pool(name="pha_late", bufs=1)
        xT_sb = pha_late.tile([128, 8, L], dt.float16)
        xT_ap = xT_d.ap().rearrange("(k p) t -> p k t", p=128)
        nc.sync.dma_start(xT_sb[:, :, 0:512], xT_ap[:, :, 0:512])

        pha = tc.alloc_tile_pool(name="pha", bufs=1)
        conv_b_sb = pha.tile([128, NBLK_F], dt.float32)
        nc.sync.dma_start(conv_b_sb, conv_b_d.ap().rearrange("(g p) -> p g", p=128))
        conv_w_sb = pha.tile([128, NBLK_F, 4], dt.float32)
        nc.sync.dma_start(conv_w_sb, conv_w_d.ap())
        dt_w_sb = pha.tile([RNK, DH], dt.float16)
        dt_b_sb = pha.tile([128, NBLK], dt.float32)
        sel_sb = pha.tile([NST, 2, 128], dt.float16)
        BT = pha.tile([NST, L], dt.float16)
        CT = pha.tile([NST, L], dt.float16)
        dtrT = pha.tile([RNK, L], dt.float16)
        xp_w_sb = pha.tile([128, NBLK_F, 128], dt.float16)
        xc = pha.tile([128, NBLK_F, L], dt.float16)

        def wxi_fetch(m):
            wxi_t = pha.tile([128, 8, 128], dt.float16, tag="wxi", bufs=3)
            nc.sync.dma_start(wxi_t, w_xi_d.ap()[m])
            return wxi_t

        # queue order: xT half 0, first two weight blocks, then the rest of
        # the bulk (block 0's matmuls only need xT[:, :, 0:512] + wxi_0)
        wxi_tiles = [wxi_fetch(0), wxi_fetch(1)]
        nc.sync.dma_start(xT_sb[:, :, 512:L], xT_ap[:, :, 512:L])
        nc.sync.dma_start(xp_w_sb, xp_w_d.ap().rearrange("(g p) j -> p g j", p=128))

        # x_proj accumulates incrementally inside the conv loop (the psum
        # group stays open across it), so dbc completes right after the last
        # conv block instead of via serial matmuls later.
        ps96_h0 = psum.tile([128, 512], dt.float32, tag="xp", bufs=2)
        ps96_h1 = psum.tile([128, 512], dt.float32, tag="xp", bufs=2)
        ps96 = [ps96_h0, ps96_h1]

        # xi blocks stream through PE; the 4-tap causal conv runs on DVE
        # (idle in phase A) as scalar_tensor_tensor chains, with ACT doing
        # the PSUM evacuation and the silu+bias.
        for m in range(NBLK_F):
            wxi_m = wxi_tiles[m]
            if m + 2 < NBLK_F:
                wxi_tiles.append(wxi_fetch(m + 2))
            xi_pad = pha.tile([128, 1028], dt.float16, tag="xi_pad", bufs=3)
            nc.vector.memset(xi_pad[:, 0:4], 0.0)
            for h in range(2):
                ps = psum.tile([128, 512], dt.float32, tag="mm")
                for k in range(8):
                    nc.tensor.matmul(
                        ps,
                        wxi_m[:, k, :],
                        xT_sb[:, k, h * 512:(h + 1) * 512],
                        start=(k == 0),
                        stop=(k == 7),
                    )
                nc.scalar.copy(xi_pad[:, 4 + h * 512: 4 + (h + 1) * 512], ps)
            # 4-tap conv on DVE as a scalar_tensor_tensor chain (the fp32
            # datapath rounds once per op, keeping the tap sum accurate)
            cacc = pha.tile([128, L], dt.float16, tag="cacc", bufs=3)
            nc.vector.tensor_scalar(
                cacc, xi_pad[:, 1:1 + L], conv_w_sb[:, m, 0:1], None, OP.mult
            )
            for j in range(1, 4):
                nc.vector.scalar_tensor_tensor(
                    cacc, xi_pad[:, 1 + j:1 + j + L], conv_w_sb[:, m, j:j + 1],
                    cacc, OP.mult, OP.add,
                )
            if sim_compat:
                sg = pha.tile([128, L], dt.float16, tag="conv_sg", bufs=3)
                nc.scalar.activation(sg, cacc, AF.Sigmoid, bias=conv_b_sb[:, m:m + 1])
                nc.vector.scalar_tensor_tensor(
                    xc[:, m, :], cacc, conv_b_sb[:, m:m + 1], sg, OP.add, OP.mult
                )
            else:
                nc.scalar.activation(
                    xc[:, m, :], cacc, AF.Silu, bias=conv_b_sb[:, m:m + 1]
                )
            for h in range(2):
                nc.tensor.matmul(
                    ps96[h],
                    xp_w_sb[:, m, :],
                    xc[:, m, h * 512:(h + 1) * 512],
                    start=(m == 0),
                    stop=(m == NBLK_F - 1),
                    skip_group_check=True,
                )
            # deferred bulk DMAs, issued mid-stream so they neither delay the
            # first xi blocks nor arrive late for their consumers
            if m == 10:
                nc.sync.dma_start(dt_w_sb, dt_w_d.ap())
                nc.sync.dma_start(dt_b_sb, dt_b_d.ap().rearrange("(g p) -> p g", p=128))
                if a_imm is None:
                    nc.sync.dma_start(A_sb, A_d.ap().rearrange("(g p) n -> p g n", p=128))
                nc.sync.dma_start(dskip_sb, dskip_d.ap().rearrange("(g p) -> p g", p=128))
                nc.sync.dma_start(ident_sb, ident_d.ap())
                nc.sync.dma_start(sel_sb, sel_d.ap())
                nc.sync.dma_start(dd_sb, dd_d.ap())

        # ---- dbc -> dt/B/C; B/C round-trip through DRAM for broadcast ----
        for h in range(2):
            nc.scalar.copy(dtrT[:, h * 512:(h + 1) * 512], ps96[h][0:RNK, :])
            nc.vector.tensor_copy(BT[:, h * 512:(h + 1) * 512], ps96[h][RNK:RNK + NST, :])
            nc.vector.tensor_copy(CT[:, h * 512:(h + 1) * 512], ps96[h][96:96 + NST, :])
        nc.sync.dma_start(bc_stage[0:NST, :], BT)
        nc.sync.dma_start(bc_stage[NST:2 * NST, :], CT)

        # y skip term D * xc: blocks 0-3 initialize y2 (DVE path); blocks 4-6
        # and 7-lo are seeded into PSUM later by diag(D) matmuls straight
        # from xc; block 7-hi is staged (its SBUF accumulator region is
        # still occupied by the bc0 broadcasts until state 1).
        for g in range(4):
            nc.vector.tensor_scalar(
                y3[:, g, :], xc[:, g, :], dskip_sb[:, g:g + 1], None, OP.mult
            )
        nc.vector.tensor_scalar(
            pad7skip, xc[:, 7, 512:L], dskip_sb[:, 7:8], None, OP.mult)

        # dt^T = softplus(dt_w^T @ dt_raw^T + dt_b), as Ln(Exp(v)+1)
        # (no Softplus table on this build; v <= ~-1 here so Exp can't
        # overflow). All 16 Exps run before the Lns: the ACT table chooser
        # reloads on every Exp<->Ln switch; batching needs 2 loads.
        # ev8 reuses xc blocks 8-15, dead once their x_proj matmuls ran.
        ev8 = xc[:, NBLK:NBLK_F, :]
        for m in range(NBLK):
            for h in range(2):
                ps = psum.tile([128, 512], dt.float32, tag="mm")
                nc.tensor.matmul(
                    ps,
                    dt_w_sb[:, m * 128:(m + 1) * 128],
                    dtrT[:, h * 512:(h + 1) * 512],
                    start=True,
                    stop=True,
                )
                nc.scalar.activation(
                    ev8[:, m, h * 512:(h + 1) * 512], ps, AF.Exp,
                    bias=dt_b_sb[:, m:m + 1],
                )
        dtT2 = dtT.rearrange("p g t -> p (g t)")
        ev8f = ev8.rearrange("p g t -> p (g t)")
        nc.scalar.activation(dtT2[:, 0:4 * L], ev8f[:, 0:4 * L], AF.Ln, bias=1.0)
        nc.scalar.activation(dtT2[:, 4 * L:8 * L], ev8f[:, 4 * L:8 * L], AF.Ln, bias=1.0)

        # u = dt * xc_half as ONE flat TT
        nc.vector.tensor_tensor(
            u2, dtT2, xc[:, 0:NBLK, :].rearrange("p g t -> p (g t)"), OP.mult)

        # pair-0 (and pair-1's first row is state 1) B/C broadcasts via PE:
        # out[p,t] = sum_k sel[k,n,p]*BT[k,t] = BT[n,t] for every partition.
        # The DMA-broadcast path used for later pairs has ~25us latency; this
        # costs 8 tiny matmuls + ACT evacs while PSUM is still free.
        for j, (row, qi) in enumerate([(BT, 0), (CT, 2), (BT, 1), (CT, 3)]):
            nst = j // 2  # B0, C0 are state 0; B1, C1 are state 1
            for h in range(2):
                ps = psum.tile([128, 512], dt.float32, tag="mm")
                nc.tensor.matmul(
                    ps, sel_sb[:, nst, :], row[:, h * 512:(h + 1) * 512],
                    start=True, stop=True,
                )
                nc.scalar.copy(bc0[:, qi, h * 512:(h + 1) * 512], ps)
            if j == 3:
                # dA(0) after all four B/C evacs: the evacs also gate
                # psum.release -> seed -> pha drain -> b0, so putting the
                # 7us exp between them would push scan 0 out by that much
                if a_imm is not None:
                    nc.scalar.activation(dA0_p, dtT2, AF.Exp, scale=float(a_imm[0]))
                else:
                    dA0_3 = dA0_p.rearrange("p (g t) -> p g t", g=NBLK)
                    for g in range(NBLK):
                        nc.scalar.activation(
                            dA0_3[:, g, :], dtT[:, g, :], AF.Exp,
                            scale=A_sb[:, g, 0:1])

        psum.release()
        # y blocks 4-6 and 7-lo accumulate in 7 PSUM banks (block 7-hi goes
        # through SBUF on DVE), leaving one bank for the z matmuls, which
        # then run concurrently with the first scan states instead of
        # blocking the whole pipeline warm-up. Seed = diag(D) @ xc.
        psumY = tc.alloc_tile_pool(name="psumY", bufs=1, space="PSUM")
        y_ps = psumY.tile([128, 7, 512], dt.float32)
        for s in range(7):
            nc.tensor.matmul(
                y_ps[:, s], dd_sb[:, s // 2, :],
                xc[:, 4 + s // 2, (s % 2) * 512:(s % 2) * 512 + 512],
                start=True, stop=False, skip_group_check=True,
            )
        psumZ = tc.alloc_tile_pool(name="psumZ", bufs=1, space="PSUM")
        pha.release()

        # ---- scan-phase + tail pools (pha space reused) ----
        phb = tc.alloc_tile_pool(name="phb", bufs=2)
        tail = tc.alloc_tile_pool(name="tail", bufs=1)

        def w2_fetch(m):
            w2m = tail.tile([128, 8, 128], dt.float16, tag="w2m", bufs=2)
            nc.sync.dma_start(w2m, w2_d.ap()[m])
            return w2m

        def bc_fetch(pair):
            # B on the SP queue, C on the ACT queue: each broadcast reads its
            # 2 source rows 128x, so one fetch is ~25us per queue; the split
            # halves it and the pair is prefetched a full state ahead anyway
            brt = tail.tile([128, 2, L], dt.float16, tag="brep", bufs=2)
            nc.sync.dma_start(
                brt, bc_stage[2 * pair:2 * pair + 2, :]
                .unsqueeze(0).broadcast_to((128, 2, L)))
            crt = tail.tile([128, 2, L], dt.float16, tag="crep", bufs=2)
            nc.scalar.dma_start(
                crt, bc_stage[NST + 2 * pair:NST + 2 * pair + 2, :]
                .unsqueeze(0).broadcast_to((128, 2, L)))
            return brt, crt

        b_t = phb.tile([128, NBLK * L], dt.float16, tag="b")
        # dA double-buffer: even states reuse the persist-pool dA0_p
        dA_b = phb.tile([128, NBLK * L], dt.float16, tag="dA", bufs=1)

        def dA_tile(n):
            return dA0_p if n % 2 == 0 else dA_b

        def dA_exp(n):
            dA_t = dA_tile(n)
            dA3 = dA_t.rearrange("p (g t) -> p g t", g=NBLK)
            if a_imm is not None:
                nc.scalar.activation(dA_t, dtT2, AF.Exp, scale=float(a_imm[n]))
            else:
                for g in range(NBLK):
                    nc.scalar.activation(
                        dA3[:, g, :], dtT[:, g, :], AF.Exp, scale=A_sb[:, g, n:n + 1]
                    )
            # reset the recurrence at each chained d-block boundary
            # (GpSimd: a DVE memset would park the in-order DVE queue on
            # this dA's exp, stalling the next scan behind it)
            nc.gpsimd.memset(dA_t[:, 0:NBLK * L:L], 0.0)

        nc.vector.tensor_tensor(
            b_t.rearrange("p (g t) -> p g t", g=NBLK), u3,
            bc0[:, 0].unsqueeze(1).broadcast_to((128, NBLK, L)), OP.mult)
        # dA0's boundary memset, after b0 so the DVE queue doesn't park on
        # the dA0 exp; dA1 next so ACT runs it ahead of the z evacuations
        nc.gpsimd.memset(dA0_p[:, 0:NBLK * L:L], 0.0)
        dA_exp(1)

        # z = x @ w_z (z^T = w_z^T @ x^T): PE + its single PSUM bank churn
        # through these in the background of the first scan states; the
        # evacuation is fused with the gate's silu in one ACT op per tile.
        for m in range(NBLK):
            wzm = pha_late.tile([128, 8, 128], dt.float16, tag="wzm", bufs=2)
            nc.sync.dma_start(wzm, w_z_d.ap()[m])
            for h in range(2):
                ps = psumZ.tile([128, 512], dt.float32, tag="zmm")
                for k in range(8):
                    nc.tensor.matmul(
                        ps,
                        wzm[:, k, :],
                        xT_sb[:, k, h * 512:(h + 1) * 512],
                        start=(k == 0),
                        stop=(k == 7),
                    )
                if sim_compat:
                    nc.scalar.copy(zT[:, m, h * 512:(h + 1) * 512], ps)
                else:
                    nc.scalar.activation(zT[:, m, h * 512:(h + 1) * 512], ps, AF.Silu)

        bc_next = bc_fetch(1)
        # first out_proj weight chunks, streamed during the scan phase
        w2_tiles = [w2_fetch(0), w2_fetch(1)]

        # ================= phase B: selective scan over n =================
        zf = zT.rearrange("p g t -> p (g t)")
        bc_cur = None
        for n in range(NST):
            if n >= 2 and n % 2 == 0:
                bc_cur = bc_next
                if n + 2 < NST:
                    bc_next = bc_fetch(n // 2 + 1)
            if n < 2:
                B_rep, C_rep = bc0[:, n], bc0[:, 2 + n]
            else:
                B_rep, C_rep = bc_cur[0][:, n % 2], bc_cur[1][:, n % 2]

            dA_t = dA_tile(n)
            h = phb.tile([128, NBLK * L], dt.float16, tag="h")
            h3 = h.rearrange("p (g t) -> p g t", g=NBLK)
            if n < NST - 1:
                nc.vector.tensor_tensor_scan(h, dA_t, b_t, 0.0, OP.mult, OP.add)
                # next state's b while this scan's consumers wait
                nb = phb.tile([128, NBLK * L], dt.float16, tag="b")
                if n == 0:
                    nB_rep = bc0[:, 1]
                else:
                    nB_rep = (bc_next if (n + 1) % 2 == 0 else bc_cur)[0][:, (n + 1) % 2]
                nc.vector.tensor_tensor(
                    nb.rearrange("p (g t) -> p g t", g=NBLK), u3,
                    nB_rep.unsqueeze(1).broadcast_to((128, NBLK, L)), OP.mult)
                nc.vector.tensor_tensor(
                    h3, h3, C_rep.unsqueeze(1).broadcast_to((128, NBLK, L)), OP.mult
                )
                nc.vector.tensor_tensor(
                    y2[:, 0:4 * L], y2[:, 0:4 * L], h[:, 0:4 * L], OP.add)
                # block 7-hi rides in SBUF (no PSUM bank left); its region is
                # bc0's C1 row, free only after state 1 consumed it
                if n == 1:
                    nc.vector.tensor_tensor(
                        y2[:, 7 * L + 512:8 * L], h_prev[:, 7 * L + 512:8 * L],
                        h[:, 7 * L + 512:8 * L], OP.add)
                elif n >= 2:
                    nc.vector.tensor_tensor(
                        y2[:, 7 * L + 512:8 * L], y2[:, 7 * L + 512:8 * L],
                        h[:, 7 * L + 512:8 * L], OP.add)
                for s in range(7):
                    nc.tensor.matmul(
                        y_ps[:, s], ident_sb,
                        h[:, 4 * L + s * 512: 4 * L + (s + 1) * 512],
                        start=False, stop=False, skip_group_check=True,
                    )
                # dA(n+2) prefetch, issued last: its DVE memset must queue
                # behind this iteration's b/C/y-add (the exp itself waits for
                # scan n to free the buffer, so an early memset would stall
                # the whole DVE queue on that exp)
                if n + 2 < NST:
                    dA_exp(n + 2)
                b_t = nb
                h_prev = h
            else:
                # last state: split the scan so the tail starts on blocks 0-3
                # while blocks 4-7 still stream through PE/PSUM
                nc.vector.tensor_tensor_scan(
                    h[:, 0:4 * L], dA_t[:, 0:4 * L], b_t[:, 0:4 * L],
                    0.0, OP.mult, OP.add)
                nc.vector.tensor_tensor_scan(
                    h[:, 4 * L:8 * L], dA_t[:, 4 * L:8 * L], b_t[:, 4 * L:8 * L],
                    0.0, OP.mult, OP.add)
                nc.vector.tensor_tensor(
                    h3[:, 0:4], h3[:, 0:4],
                    C_rep.unsqueeze(1).broadcast_to((128, 4, L)), OP.mult)
                nc.vector.tensor_tensor(
                    y2[:, 0:4 * L], y2[:, 0:4 * L], h[:, 0:4 * L], OP.add)
                if not sim_compat:
                    # gate blocks 0-3 in place: y2 *= silu(z)
                    nc.vector.tensor_tensor(
                        y2[:, 0:4 * L], y2[:, 0:4 * L], zf[:, 0:4 * L], OP.mult)
                nc.vector.tensor_tensor(
                    h3[:, 4:8], h3[:, 4:8],
                    C_rep.unsqueeze(1).broadcast_to((128, 4, L)), OP.mult)
                nc.vector.tensor_tensor(
                    y2[:, 7 * L + 512:8 * L], y2[:, 7 * L + 512:8 * L],
                    h[:, 7 * L + 512:8 * L], OP.add)
                for s in range(7):
                    nc.tensor.matmul(
                        y_ps[:, s], ident_sb,
                        h[:, 4 * L + s * 512: 4 * L + (s + 1) * 512],
                        start=False, stop=True, skip_group_check=True,
                    )
                nc.vector.tensor_tensor(
                    y2[:, 7 * L + 512:8 * L], y2[:, 7 * L + 512:8 * L],
                    pad7skip, OP.add)
                if not sim_compat:
                    # gate blocks 4-7lo straight from PSUM into dead u2
                    # space, block 7-hi from its SBUF accumulator
                    nc.vector.tensor_tensor(
                        u2[:, 0:3 * L + 512],
                        y_ps.rearrange("p s t -> p (s t)"),
                        zf[:, 4 * L:7 * L + 512], OP.mult)
                    nc.vector.tensor_tensor(
                        u2[:, 3 * L + 512:4 * L], y2[:, 7 * L + 512:8 * L],
                        zf[:, 7 * L + 512:8 * L], OP.mult)

        if sim_compat:
            sg = phb.tile([128, NBLK * L], dt.float16, tag="h")
            sg3 = sg.rearrange("p (g t) -> p g t", g=NBLK)
            for g in range(NBLK):
                nc.scalar.activation(sg3[:, g, :], zT[:, g, :], AF.Sigmoid)
            nc.vector.tensor_tensor(sg, sg, zf, OP.mult)
            nc.vector.tensor_tensor(
                y2[:, 0:4 * L], y2[:, 0:4 * L], sg[:, 0:4 * L], OP.mult)
            nc.vector.tensor_tensor(
                u2[:, 0:3 * L + 512],
                y_ps.rearrange("p s t -> p (s t)"),
                sg[:, 4 * L:7 * L + 512], OP.mult)
            nc.vector.tensor_tensor(
                u2[:, 3 * L + 512:4 * L], y2[:, 7 * L + 512:8 * L],
                sg[:, 7 * L + 512:8 * L], OP.mult)
        psumZ.release()
        psumY.release()

        # ========= phase C: fused (out_proj @ proj) matmul =========
        # gated y lives in y2[0:4L] (blocks 0-3) and u2[0:4L] (blocks 4-7)
        psumC = tc.alloc_tile_pool(name="psumC", bufs=6, space="PSUM")
        pT_ap = pT_d.ap().rearrange("(k p) t -> p k t", p=128)
        for m in range(8):
            w2m = w2_tiles[m]
            if m + 2 < 8:
                w2_tiles.append(w2_fetch(m + 2))
            stg = tail.tile([128, L], dt.float16, tag="stg", bufs=2)
            ps = psumC.tile([128, L], dt.float32, tag="mm2", bufs=3)
            for h in range(2):
                for k in range(8):
                    src = y2 if k < 4 else u2
                    off = (k if k < 4 else k - 4) * L
                    nc.tensor.matmul(
                        ps[:, h * 512:(h + 1) * 512],
                        w2m[:, k, :],
                        src[:, off + h * 512: off + (h + 1) * 512],
                        start=(k == 0),
                        stop=(k == 7),
                    )
            nc.vector.tensor_copy(stg, ps)
            nc.sync.dma_start(pT_ap[:, m, :], stg)
        psumC.release()
        tail.release()
        phb.release()
        pha_late.release()
        dram.release()
        persist.release()
        const.release()

    nc.compile()
    return nc


def _conv_taps(conv_w):
    """(DI, 4) -> (128, 16, 4) fp32: [p, g, j] = conv_w[g*128+p, j]."""
    return np.ascontiguousarray(
        conv_w.reshape(NBLK_F, 128, 4).transpose(1, 0, 2), dtype=np.float32)


def _wxi_layout(w):
    """(D, nm*128) -> (nm, 128, 8, 128): [m, p, k, c] = w[k*128+p, m*128+c]
    so each m-block DMA reads contiguous 2KB per partition."""
    nm = w.shape[1] // 128
    return np.ascontiguousarray(
        w.reshape(8, 128, nm, 128).transpose(2, 1, 0, 3), dtype=F16)


def _a_imm(inputs):
    """If A = -exp(A_log) is identical across d and across all cores' slices,
    return the 16 per-state values to bake as immediates, else None."""
    al = np.float64(inputs["A_log"])
    A = (-np.exp(al)).astype(np.float32)       # (2, DI, NST)
    row = A[0, 0]
    if np.array_equal(A, np.broadcast_to(row, A.shape)):
        return tuple(float(v) for v in row)
    return None


def _prep_core_inputs(inputs, c, with_A):
    """Slice/permute/cast the full inputs for core c (all numpy, cheap)."""
    dr, b, half = c // 4, (c // 2) % 2, c % 2
    s0 = half * DH
    # d_inner permutation putting this core's half first
    perm = np.r_[DH:DI, 0:DH] if half == 1 else np.r_[0:DI]

    x = inputs["x"][b]
    if dr == 1:
        x = x[::-1]
    in_w = inputs["in_w"][dr]

    m = {
        "xT": np.ascontiguousarray(x.T, dtype=F16),
        "w_xi": _wxi_layout(in_w[:, :DI][:, perm]),
        "w_z": _wxi_layout(in_w[:, DI + s0:DI + s0 + DH]),
        "conv_w": _conv_taps(inputs["conv_w"][dr][perm]),
        "conv_b": np.ascontiguousarray(inputs["conv_b"][dr][perm], dtype=np.float32),
        "xp_w": _pad_xp(inputs["xp_w"][dr][perm]),
        "dt_w": np.ascontiguousarray(inputs["dt_w"][dr][:, s0:s0 + DH], dtype=F16),
        "dt_b": np.ascontiguousarray(inputs["dt_b"][dr][s0:s0 + DH], dtype=np.float32),
        "dskip": np.ascontiguousarray(inputs["D"][dr][s0:s0 + DH], dtype=np.float32),
        # fused (out_w_half @ proj_w_dir): one matmul on-device instead of two
        "w2": _wxi_layout(
            inputs["out_w"][dr][s0:s0 + DH].astype(np.float32)
            @ inputs["proj_w"][dr * D:(dr + 1) * D].astype(np.float32)),
        "ident": np.eye(128, dtype=F16),
        "sel": _sel_rows(),
        "dskip_diag": _dskip_diag(inputs["D"][dr][s0:s0 + DH]),
    }
    if with_A:
        A_full = -np.exp(np.float64(inputs["A_log"][dr])).astype(np.float32)
        m["A"] = np.ascontiguousarray(A_full[s0:s0 + DH], dtype=np.float32)
    return m


def _dskip_diag(dskip):
    """(DH,) -> (128, 4, 128) diag(D) for d-blocks 4..7 (PSUM y seed)."""
    out = np.zeros((128, 4, 128), F16)
    idx = np.arange(128)
    for i in range(4):
        out[idx, i, idx] = dskip[(4 + i) * 128:(5 + i) * 128].astype(F16)
    return out


def _sel_rows():
    """(NST, 2, 128) one-hot-row selectors: sel[k, j, :] = (k == j)."""
    out = np.zeros((NST, 2, 128), F16)
    out[0, 0, :] = 1.0
    out[1, 1, :] = 1.0
    return out


def _pad_xp(xp):
    """(DI, 96) -> (DI, 128) with C cols moved to 96 (PSUM partition-start
    alignment: compute engines can only read partitions starting at 0/32/64/96)."""
    out = np.zeros((DI, 128), F16)
    out[:, :RNK + NST] = xp[:, :RNK + NST]
    out[:, 96:96 + NST] = xp[:, RNK + NST:]
    return out


def _gather(inputs, results):
    out = np.zeros((B, L, D), np.float32)
    for c, res in enumerate(results):
        dr, b = c // 4, (c // 2) % 2
        p = res["pT"].astype(np.float32).T
        if dr == 1:
            p = p[::-1]
        out[b] += p
    out += inputs["proj_b"]
    return out


def kernel(**inputs):
    inputs = {k: np.asarray(v) for k, v in inputs.items()}
    a_imm = _a_imm(inputs)
    key = ("nc", a_imm)
    if key not in _CACHE:
        _CACHE[key] = _build_module(a_imm=a_imm)
    nc = _CACHE[key]
    in_maps = [_prep_core_inputs(inputs, c, with_A=a_imm is None) for c in range(8)]
    from concourse.bass_utils import run_bass_kernel_spmd
    res = run_bass_kernel_spmd(nc, in_maps, core_ids=list(range(8)))
    return _gather(inputs, res.results)


# revision 65
# speedup vs baseline: 1.0019x; 1.0019x over previous
"""BiMamba block on 8 Trainium2 NeuronCores via Bass/Tile.

Sharding (SPMD, one shared NEFF, no collectives):
  core c: dir = c//4 (0=fwd, 1=bwd), batch = (c//2)%2, half = c%2.
Each core runs the full mamba pipeline for one (dir, batch) pair on its
half of d_inner (scan channels are independent), computing the full-d_inner
xi/conv/x_proj path locally (dt/B/C need the full d_inner contraction).
The d_inner axis is permuted per core so its own half is always blocks 0..7,
keeping the program identical across cores. Each core emits a partial
output (d_model, L) = (y_half @ out_w_half) @ proj_w_dir, transposed;
the host sums the 8 partials, un-reverses the bwd direction, adds proj_b.

Layouts: everything on-chip is "transposed" (feature dim on partitions,
time on the free axis) so the causal conv is a free-dim shift, the scan
runs along the free axis (DVE tensor_tensor_scan), and every matmul uses
naturally-laid-out weights as the stationary lhsT operand.

v2 engine split (the DVE scan is the hard floor: 2 cyc/elem dependency-
bound, 17.2us per state, 16 states back-to-back = ~437us of the runtime):
  PE:  in_proj xi, x_proj, dt, selector/diag(D) broadcasts, z, identity
       y-accumulate, out_proj.
  DVE: causal conv (per-tap scalar_tensor_tensor), u/b/C mults, scan, y-add.
  ACT: xi PSUM evacuation, silu(+bias), softplus (Exp+Ln), dA exps, fused
       z silu-evacuation.
Per state n: dA = exp(A[:,n]*dt) (ACT, double-buffered one state ahead),
b = u*B_n (DVE, partition-broadcast B), h = scan(dA, b) with chain-reset
via GpSimd dA=0 memsets at block boundaries (a DVE memset would park the
in-order DVE queue on the exp), y += h*C_n.

y accumulation: blocks 0-3 DVE-add in SBUF fp16; blocks 4-6 + 7-lo via PE
identity matmuls into 7 PSUM banks; block 7-hi DVE-adds into y2's dead
second half. The 8th PSUM bank runs the z matmuls DURING the early scan
states. Pool-overlap deps are pool-granular (an alloc over a released zone
waits for the WHOLE old pool), so x^T + streamed w_z chunks live in their
own never-overlapped pool (pha_late) and the rest of phase A (pha) is
released right after u/seed, letting scan 0 start at ~129us.

B/C rows: states 0/1 are broadcast by PE one-hot-row selector matmuls
(the DMA broadcast has ~25us latency); later pairs are DMA-broadcast from
a DRAM staging buffer, prefetched one pair ahead on both HWDGE queues.
State 15 is split into two half scans and the output partial is fp16 so
the tail (gate + fused out_w@proj_w matmul, 2-bank PSUM groups, streamed
weights) finishes ~56us after the last scan starts.
"""

import numpy as np

B, L, D = 2, 1024, 1024
DI, DH, NST, RNK = 2048, 1024, 16, 64
NBLK = DH // 128          # 8 d-blocks per half
NBLK_F = DI // 128        # 16 d-blocks full
F16 = np.float16

_CACHE = {}


def _build_module(sim_compat=False, a_imm=None):
    """sim_compat=True replaces Silu (absent from CoreSim) with
    Sigmoid + multiply; the hardware build uses the Silu table directly."""
    import concourse.bass as bass
    import concourse.mybir as mybir
    from concourse import bacc
    from concourse.tile import TileContext

    dt = mybir.dt
    AF = mybir.ActivationFunctionType
    OP = mybir.AluOpType

    nc = bacc.Bacc("TRN2", target_bir_lowering=False, debug=False)

    # ---- DRAM I/O ----
    xT_d = nc.dram_tensor("xT", (D, L), dt.float16, kind="ExternalInput")
    w_xi_d = nc.dram_tensor("w_xi", (NBLK_F, 128, 8, 128), dt.float16, kind="ExternalInput")
    w_z_d = nc.dram_tensor("w_z", (8, 128, 8, 128), dt.float16, kind="ExternalInput")
    conv_w_d = nc.dram_tensor("conv_w", (128, NBLK_F, 4), dt.float32, kind="ExternalInput")
    conv_b_d = nc.dram_tensor("conv_b", (DI,), dt.float32, kind="ExternalInput")
    xp_w_d = nc.dram_tensor("xp_w", (DI, 128), dt.float16, kind="ExternalInput")
    dt_w_d = nc.dram_tensor("dt_w", (RNK, DH), dt.float16, kind="ExternalInput")
    dt_b_d = nc.dram_tensor("dt_b", (DH,), dt.float32, kind="ExternalInput")
    A_d = None
    if a_imm is None:
        A_d = nc.dram_tensor("A", (DH, NST), dt.float32, kind="ExternalInput")
    dskip_d = nc.dram_tensor("dskip", (DH,), dt.float32, kind="ExternalInput")
    w2_d = nc.dram_tensor("w2", (8, 128, 8, 128), dt.float16, kind="ExternalInput")
    ident_d = nc.dram_tensor("ident", (128, 128), dt.float16, kind="ExternalInput")
    # one-hot-row selectors: sel[k, j, :] = 1.0 iff k == j (j = state 0/1)
    sel_d = nc.dram_tensor("sel", (NST, 2, 128), dt.float16, kind="ExternalInput")
    # diag(D) per d-block 4..7, the PSUM seed weights for the y skip term
    dd_d = nc.dram_tensor("dskip_diag", (128, 4, 128), dt.float16, kind="ExternalInput")
    pT_d = nc.dram_tensor("pT", (D, L), dt.float16, kind="ExternalOutput")

    with TileContext(nc) as tc:
        psum = tc.alloc_tile_pool(name="psum", bufs=6, space="PSUM")
        const = tc.alloc_tile_pool(name="const", bufs=1)
        persist = tc.alloc_tile_pool(name="persist", bufs=1)
        dram = tc.alloc_tile_pool(name="dram", bufs=1, space="DRAM")
        # B/C rows staged in DRAM so they can be partition-broadcast by DMA
        bc_stage = dram.tile([2 * NST, L], dt.float16)

        # ---- constants read during the scan phase (everything phase-A-only
        # lives in pha_early so its space is reclaimed for the scan pools) ----
        A_sb = None
        if a_imm is None:
            A_sb = const.tile([128, NBLK, NST], dt.float32)
        dskip_sb = const.tile([128, NBLK], dt.float32)
        ident_sb = const.tile([128, 128], dt.float16)
        dd_sb = const.tile([128, 4, 128], dt.float16)

        # ---- persistent activations ----
        zT = persist.tile([128, NBLK, L], dt.float16)
        dtT = persist.tile([128, NBLK, L], dt.float16)
        u2 = persist.tile([128, NBLK * L], dt.float16)
        y2 = persist.tile([128, NBLK * L], dt.float16)
        u3 = u2.rearrange("p (g t) -> p g t", g=NBLK)
        y3 = y2.rearrange("p (g t) -> p g t", g=NBLK)
        # pair-0 B/C broadcasts (PE selector matmuls) alias y2 blocks 4-7,
        # which accumulate in PSUM and never touch their SBUF half; dA(0)
        # needs the persist pool (both are produced while the phase A pools
        # still hold all of SBUF, so they cannot live in phb)
        bc0 = y2[:, 4 * L:8 * L].rearrange("p (q t) -> p q t", q=4)
        dA0_p = persist.tile([128, NBLK * L], dt.float16)
        # block 7 upper-half skip term D*xc, staged until its y accumulator
        # region (the tail of bc0) frees up after state 1
        pad7skip = persist.tile([128, 512], dt.float16)

        # ================= phase A: in_proj, conv, x_proj, dt =================
        # Pool-overlap dependencies are pool-granular: a pool allocated over
        # a released zone waits for the WHOLE old pool to drain. The z
        # matmuls keep reading x^T until deep into the scan phase, so x^T
        # (plus the streamed w_z chunks) lives in its own pool, kept alive
        # until the end; everything else phase-A is in pha_early, released
        # right after dbc/u so the scan pools start immediately.
        pha_late = tc.alloc_tile_pool(name="pha_late", bufs=1)
        xT_sb = pha_late.tile([128, 8, L], dt.float16)
        xT_ap = xT_d.ap().rearrange("(k p) t -> p k t", p=128)
        nc.sync.dma_start(xT_sb[:, :, 0:512], xT_ap[:, :, 0:512])

        pha = tc.alloc_tile_pool(name="pha", bufs=1)
        conv_b_sb = pha.tile([128, NBLK_F], dt.float32)
        nc.sync.dma_start(conv_b_sb, conv_b_d.ap().rearrange("(g p) -> p g", p=128))
        conv_w_sb = pha.tile([128, NBLK_F, 4], dt.float32)
        nc.sync.dma_start(conv_w_sb, conv_w_d.ap())
        dt_w_sb = pha.tile([RNK, DH], dt.float16)
        dt_b_sb = pha.tile([128, NBLK], dt.float32)
        sel_sb = pha.tile([NST, 2, 128], dt.float16)
        BT = pha.tile([NST, L], dt.float16)
        CT = pha.tile([NST, L], dt.float16)
        dtrT = pha.tile([RNK, L], dt.float16)
        xp_w_sb = pha.tile([128, NBLK_F, 128], dt.float16)
        xc = pha.tile([128, NBLK_F, L], dt.float16)

        def wxi_fetch(m):
            wxi_t = pha.tile([128, 8, 128], dt.float16, tag="wxi", bufs=3)
            nc.sync.dma_start(wxi_t, w_xi_d.ap()[m])
            return wxi_t

        # queue order: xT half 0, first two weight blocks, then the rest of
        # the bulk (block 0's matmuls only need xT[:, :, 0:512] + wxi_0)
        wxi_tiles = [wxi_fetch(0), wxi_fetch(1)]
        nc.sync.dma_start(xT_sb[:, :, 512:L], xT_ap[:, :, 512:L])
        nc.sync.dma_start(xp_w_sb, xp_w_d.ap().rearrange("(g p) j -> p g j", p=128))

        # x_proj accumulates incrementally inside the conv loop (the psum
        # group stays open across it), so dbc completes right after the last
        # conv block instead of via serial matmuls later.
        ps96_h0 = psum.tile([128, 512], dt.float32, tag="xp", bufs=2)
        ps96_h1 = psum.tile([128, 512], dt.float32, tag="xp", bufs=2)
        ps96 = [ps96_h0, ps96_h1]

        # xi blocks stream through PE; the 4-tap causal conv runs on DVE
        # (idle in phase A) as scalar_tensor_tensor chains, with ACT doing
        # the PSUM evacuation and the silu+bias.
        for m in range(NBLK_F):
            wxi_m = wxi_tiles[m]
            if m + 2 < NBLK_F:
                wxi_tiles.append(wxi_fetch(m + 2))
            xi_pad = pha.tile([128, 1028], dt.float16, tag="xi_pad", bufs=3)
            nc.vector.memset(xi_pad[:, 0:4], 0.0)
            for h in range(2):
                ps = psum.tile([128, 512], dt.float32, tag="mm")
                for k in range(8):
                    nc.tensor.matmul(
                        ps,
                        wxi_m[:, k, :],
                        xT_sb[:, k, h * 512:(h + 1) * 512],
                        start=(k == 0),
                        stop=(k == 7),
                    )
                nc.scalar.copy(xi_pad[:, 4 + h * 512: 4 + (h + 1) * 512], ps)
            # 4-tap conv on DVE as a scalar_tensor_tensor chain (the fp32
            # datapath rounds once per op, keeping the tap sum accurate)
            cacc = pha.tile([128, L], dt.float16, tag="cacc", bufs=3)
            nc.vector.tensor_scalar(
                cacc, xi_pad[:, 1:1 + L], conv_w_sb[:, m, 0:1], None, OP.mult
            )
            for j in range(1, 4):
                nc.vector.scalar_tensor_tensor(
                    cacc, xi_pad[:, 1 + j:1 + j + L], conv_w_sb[:, m, j:j + 1],
                    cacc, OP.mult, OP.add,
                )
            if sim_compat:
                sg = pha.tile([128, L], dt.float16, tag="conv_sg", bufs=3)
                nc.scalar.activation(sg, cacc, AF.Sigmoid, bias=conv_b_sb[:, m:m + 1])
                nc.vector.scalar_tensor_tensor(
                    xc[:, m, :], cacc, conv_b_sb[:, m:m + 1], sg, OP.add, OP.mult
                )
            else:
                nc.scalar.activation(
                    xc[:, m, :], cacc, AF.Silu, bias=conv_b_sb[:, m:m + 1]
                )
            for h in range(2):
                nc.tensor.matmul(
                    ps96[h],
                    xp_w_sb[:, m, :],
                    xc[:, m, h * 512:(h + 1) * 512],
                    start=(m == 0),
                    stop=(m == NBLK_F - 1),
                    skip_group_check=True,
                )
            # deferred bulk DMAs, issued mid-stream so they neither delay the
            # first xi blocks nor arrive late for their consumers
            if m == 10:
                nc.sync.dma_start(dt_w_sb, dt_w_d.ap())
                nc.sync.dma_start(dt_b_sb, dt_b_d.ap().rearrange("(g p) -> p g", p=128))
                if a_imm is None:
                    nc.sync.dma_start(A_sb, A_d.ap().rearrange("(g p) n -> p g n", p=128))
                nc.sync.dma_start(dskip_sb, dskip_d.ap().rearrange("(g p) -> p g", p=128))
                nc.sync.dma_start(ident_sb, ident_d.ap())
                nc.sync.dma_start(sel_sb, sel_d.ap())
                nc.sync.dma_start(dd_sb, dd_d.ap())

        # ---- dbc -> dt/B/C; B/C round-trip through DRAM for broadcast ----
        for h in range(2):
            nc.scalar.copy(dtrT[:, h * 512:(h + 1) * 512], ps96[h][0:RNK, :])
            nc.vector.tensor_copy(BT[:, h * 512:(h + 1) * 512], ps96[h][RNK:RNK + NST, :])
            nc.vector.tensor_copy(CT[:, h * 512:(h + 1) * 512], ps96[h][96:96 + NST, :])
        nc.sync.dma_start(bc_stage[0:NST, :], BT)
        nc.sync.dma_start(bc_stage[NST:2 * NST, :], CT)

        # y skip term D * xc: blocks 0-3 initialize y2 (DVE path); blocks 4-6
        # and 7-lo are seeded into PSUM later by diag(D) matmuls straight
        # from xc; block 7-hi is staged (its SBUF accumulator region is
        # still occupied by the bc0 broadcasts until state 1).
        for g in range(4):
            nc.vector.tensor_scalar(
                y3[:, g, :], xc[:, g, :], dskip_sb[:, g:g + 1], None, OP.mult
            )
        nc.vector.tensor_scalar(
            pad7skip, xc[:, 7, 512:L], dskip_sb[:, 7:8], None, OP.mult)

        # dt^T = softplus(dt_w^T @ dt_raw^T + dt_b), as Ln(Exp(v)+1)
        # (no Softplus table on this build; v <= ~-1 here so Exp can't
        # overflow). All 16 Exps run before the Lns: the ACT table chooser
        # reloads on every Exp<->Ln switch; batching needs 2 loads.
        # ev8 reuses xc blocks 8-15, dead once their x_proj matmuls ran.
        ev8 = xc[:, NBLK:NBLK_F, :]
        for m in range(NBLK):
            for h in range(2):
                ps = psum.tile([128, 512], dt.float32, tag="mm")
                nc.tensor.matmul(
                    ps,
                    dt_w_sb[:, m * 128:(m + 1) * 128],
                    dtrT[:, h * 512:(h + 1) * 512],
                    start=True,
                    stop=True,
                )
                nc.scalar.activation(
                    ev8[:, m, h * 512:(h + 1) * 512], ps, AF.Exp,
                    bias=dt_b_sb[:, m:m + 1],
                )
        dtT2 = dtT.rearrange("p g t -> p (g t)")
        ev8f = ev8.rearrange("p g t -> p (g t)")
        nc.scalar.activation(dtT2[:, 0:4 * L], ev8f[:, 0:4 * L], AF.Ln, bias=1.0)
        nc.scalar.activation(dtT2[:, 4 * L:8 * L], ev8f[:, 4 * L:8 * L], AF.Ln, bias=1.0)

        # u = dt * xc_half as ONE flat TT
        nc.vector.tensor_tensor(
            u2, dtT2, xc[:, 0:NBLK, :].rearrange("p g t -> p (g t)"), OP.mult)

        # pair-0 (and pair-1's first row is state 1) B/C broadcasts via PE:
        # out[p,t] = sum_k sel[k,n,p]*BT[k,t] = BT[n,t] for every partition.
        # The DMA-broadcast path used for later pairs has ~25us latency; this
        # costs 8 tiny matmuls + ACT evacs while PSUM is still free.
        for j, (row, qi) in enumerate([(BT, 0), (CT, 2), (BT, 1), (CT, 3)]):
            nst = j // 2  # B0, C0 are state 0; B1, C1 are state 1
            for h in range(2):
                ps = psum.tile([128, 512], dt.float32, tag="mm")
                nc.tensor.matmul(
                    ps, sel_sb[:, nst, :], row[:, h * 512:(h + 1) * 512],
                    start=True, stop=True,
                )
                nc.scalar.copy(bc0[:, qi, h * 512:(h + 1) * 512], ps)
            if j == 3:
                # dA(0) after all four B/C evacs: the evacs also gate
                # psum.release -> seed -> pha drain -> b0, so putting the
                # 7us exp between them would push scan 0 out by that much
                if a_imm is not None:
                    nc.scalar.activation(dA0_p, dtT2, AF.Exp, scale=float(a_imm[0]))
                else:
                    dA0_3 = dA0_p.rearrange("p (g t) -> p g t", g=NBLK)
                    for g in range(NBLK):
                        nc.scalar.activation(
                            dA0_3[:, g, :], dtT[:, g, :], AF.Exp,
                            scale=A_sb[:, g, 0:1])

        psum.release()
        # y blocks 4-6 and 7-lo accumulate in 7 PSUM banks (block 7-hi goes
        # through SBUF on DVE), leaving one bank for the z matmuls, which
        # then run concurrently with the first scan states instead of
        # blocking the whole pipeline warm-up. Seed = diag(D) @ xc.
        psumY = tc.alloc_tile_pool(name="psumY", bufs=1, space="PSUM")
        y_ps = psumY.tile([128, 7, 512], dt.float32)
        for s in range(7):
            nc.tensor.matmul(
                y_ps[:, s], dd_sb[:, s // 2, :],
                xc[:, 4 + s // 2, (s % 2) * 512:(s % 2) * 512 + 512],
                start=True, stop=False, skip_group_check=True,
            )
        psumZ = tc.alloc_tile_pool(name="psumZ", bufs=1, space="PSUM")
        pha.release()

        # ---- scan-phase + tail pools (pha space reused) ----
        phb = tc.alloc_tile_pool(name="phb", bufs=2)
        tail = tc.alloc_tile_pool(name="tail", bufs=1)

        def w2_fetch(m):
            w2m = tail.tile([128, 8, 128], dt.float16, tag="w2m", bufs=2)
            nc.sync.dma_start(w2m, w2_d.ap()[m])
            return w2m

        def bc_fetch(pair):
            # B on the SP queue, C on the ACT queue: each broadcast reads its
            # 2 source rows 128x, so one fetch is ~25us per queue; the split
            # halves it and the pair is prefetched a full state ahead anyway
            brt = tail.tile([128, 2, L], dt.float16, tag="brep", bufs=2)
            nc.sync.dma_start(
                brt, bc_stage[2 * pair:2 * pair + 2, :]
                .unsqueeze(0).broadcast_to((128, 2, L)))
            crt = tail.tile([128, 2, L], dt.float16, tag="crep", bufs=2)
            nc.scalar.dma_start(
                crt, bc_stage[NST + 2 * pair:NST + 2 * pair + 2, :]
                .unsqueeze(0).broadcast_to((128, 2, L)))
            return brt, crt

        b_t = phb.tile([128, NBLK * L], dt.float16, tag="b")
        # dA double-buffer: even states reuse the persist-pool dA0_p
        dA_b = phb.tile([128, NBLK * L], dt.float16, tag="dA", bufs=1)

        def dA_tile(n):
            return dA0_p if n % 2 == 0 else dA_b

        def dA_exp(n):
            dA_t = dA_tile(n)
            dA3 = dA_t.rearrange("p (g t) -> p g t", g=NBLK)
            if a_imm is not None:
                nc.scalar.activation(dA_t, dtT2, AF.Exp, scale=float(a_imm[n]))
            else:
                for g in range(NBLK):
                    nc.scalar.activation(
                        dA3[:, g, :], dtT[:, g, :], AF.Exp, scale=A_sb[:, g, n:n + 1]
                    )
            # reset the recurrence at each chained d-block boundary
            # (GpSimd: a DVE memset would park the in-order DVE queue on
            # this dA's exp, stalling the next scan behind it)
            nc.gpsimd.memset(dA_t[:, 0:NBLK * L:L], 0.0)

        nc.vector.tensor_tensor(
            b_t.rearrange("p (g t) -> p g t", g=NBLK), u3,
            bc0[:, 0].unsqueeze(1).broadcast_to((128, NBLK, L)), OP.mult)
        # dA0's boundary memset, after b0 so the DVE queue doesn't park on
        # the dA0 exp; dA1 next so ACT runs it ahead of the z evacuations
        nc.gpsimd.memset(dA0_p[:, 0:NBLK * L:L], 0.0)
        dA_exp(1)

        # z = x @ w_z (z^T = w_z^T @ x^T): PE + its single PSUM bank churn
        # through these in the background of the first scan states; the
        # evacuation is fused with the gate's silu in one ACT op per tile.
        for m in range(NBLK):
            wzm = pha_late.tile([128, 8, 128], dt.float16, tag="wzm", bufs=2)
            nc.sync.dma_start(wzm, w_z_d.ap()[m])
            for h in range(2):
                ps = psumZ.tile([128, 512], dt.float32, tag="zmm")
                for k in range(8):
                    nc.tensor.matmul(
                        ps,
                        wzm[:, k, :],
                        xT_sb[:, k, h * 512:(h + 1) * 512],
                        start=(k == 0),
                        stop=(k == 7),
                    )
                if sim_compat:
                    nc.scalar.copy(zT[:, m, h * 512:(h + 1) * 512], ps)
                else:
                    nc.scalar.activation(zT[:, m, h * 512:(h + 1) * 512], ps, AF.Silu)

        bc_next = bc_fetch(1)
        # first out_proj weight chunks, streamed during the scan phase
        w2_tiles = [w2_fetch(0), w2_fetch(1)]

        # ================= phase B: selective scan over n =================
        zf = zT.rearrange("p g t -> p (g t)")
        bc_cur = None
        for n in range(NST):
            if n >= 2 and n % 2 == 0:
                bc_cur = bc_next
                if n + 2 < NST:
                    bc_next = bc_fetch(n // 2 + 1)
            if n < 2:
                B_rep, C_rep = bc0[:, n], bc0[:, 2 + n]
            else:
                B_rep, C_rep = bc_cur[0][:, n % 2], bc_cur[1][:, n % 2]

            dA_t = dA_tile(n)
            h = phb.tile([128, NBLK * L], dt.float16, tag="h")
            h3 = h.rearrange("p (g t) -> p g t", g=NBLK)
            if n < NST - 1:
                nc.vector.tensor_tensor_scan(h, dA_t, b_t, 0.0, OP.mult, OP.add)
                # next state's b while this scan's consumers wait
                nb = phb.tile([128, NBLK * L], dt.float16, tag="b")
                if n == 0:
                    nB_rep = bc0[:, 1]
                else:
                    nB_rep = (bc_next if (n + 1) % 2 == 0 else bc_cur)[0][:, (n + 1) % 2]
                nc.vector.tensor_tensor(
                    nb.rearrange("p (g t) -> p g t", g=NBLK), u3,
                    nB_rep.unsqueeze(1).broadcast_to((128, NBLK, L)), OP.mult)
                nc.vector.tensor_tensor(
                    h3, h3, C_rep.unsqueeze(1).broadcast_to((128, NBLK, L)), OP.mult
                )
                nc.vector.tensor_tensor(
                    y2[:, 0:4 * L], y2[:, 0:4 * L], h[:, 0:4 * L], OP.add)
                # block 7-hi rides in SBUF (no PSUM bank left); its region is
                # bc0's C1 row, free only after state 1 consumed it
                if n == 1:
                    nc.vector.tensor_tensor(
                        y2[:, 7 * L + 512:8 * L], h_prev[:, 7 * L + 512:8 * L],
                        h[:, 7 * L + 512:8 * L], OP.add)
                elif n >= 2:
                    nc.vector.tensor_tensor(
                        y2[:, 7 * L + 512:8 * L], y2[:, 7 * L + 512:8 * L],
                        h[:, 7 * L + 512:8 * L], OP.add)
                for s in range(7):
                    nc.tensor.matmul(
                        y_ps[:, s], ident_sb,
                        h[:, 4 * L + s * 512: 4 * L + (s + 1) * 512],
                        start=False, stop=False, skip_group_check=True,
                    )
                # dA(n+2) prefetch, issued last: its DVE memset must queue
                # behind this iteration's b/C/y-add (the exp itself waits for
                # scan n to free the buffer, so an early memset would stall
                # the whole DVE queue on that exp)
                if n + 2 < NST:
                    dA_exp(n + 2)
                b_t = nb
                h_prev = h
            else:
                # last state: split the scan so the tail starts on blocks 0-3
                # while blocks 4-7 still stream through PE/PSUM
                nc.vector.tensor_tensor_scan(
                    h[:, 0:4 * L], dA_t[:, 0:4 * L], b_t[:, 0:4 * L],
                    0.0, OP.mult, OP.add)
                nc.vector.tensor_tensor_scan(
                    h[:, 4 * L:8 * L], dA_t[:, 4 * L:8 * L], b_t[:, 4 * L:8 * L],
                    0.0, OP.mult, OP.add)
                nc.vector.tensor_tensor(
                    h3[:, 0:4], h3[:, 0:4],
                    C_rep.unsqueeze(1).broadcast_to((128, 4, L)), OP.mult)
                nc.vector.tensor_tensor(
                    y2[:, 0:4 * L], y2[:, 0:4 * L], h[:, 0:4 * L], OP.add)
                if not sim_compat:
                    # gate blocks 0-3 in place: y2 *= silu(z)
                    nc.vector.tensor_tensor(
                        y2[:, 0:4 * L], y2[:, 0:4 * L], zf[:, 0:4 * L], OP.mult)
                nc.vector.tensor_tensor(
                    h3[:, 4:8], h3[:, 4:8],
                    C_rep.unsqueeze(1).broadcast_to((128, 4, L)), OP.mult)
                nc.vector.tensor_tensor(
                    y2[:, 7 * L + 512:8 * L], y2[:, 7 * L + 512:8 * L],
                    h[:, 7 * L + 512:8 * L], OP.add)
                for s in range(7):
                    nc.tensor.matmul(
                        y_ps[:, s], ident_sb,
                        h[:, 4 * L + s * 512: 4 * L + (s + 1) * 512],
                        start=False, stop=True, skip_group_check=True,
                    )
                nc.vector.tensor_tensor(
                    y2[:, 7 * L + 512:8 * L], y2[:, 7 * L + 512:8 * L],
                    pad7skip, OP.add)
                if not sim_compat:
                    # gate blocks 4-7lo straight from PSUM into dead u2
                    # space, block 7-hi from its SBUF accumulator
                    nc.vector.tensor_tensor(
                        u2[:, 0:3 * L + 512],
                        y_ps.rearrange("p s t -> p (s t)"),
                        zf[:, 4 * L:7 * L + 512], OP.mult)
                    nc.vector.tensor_tensor(
                        u2[:, 3 * L + 512:4 * L], y2[:, 7 * L + 512:8 * L],
                        zf[:, 7 * L + 512:8 * L], OP.mult)

        if sim_compat:
            sg = phb.tile([128, NBLK * L], dt.float16, tag="h")
            sg3 = sg.rearrange("p (g t) -> p g t", g=NBLK)
            for g in range(NBLK):
                nc.scalar.activation(sg3[:, g, :], zT[:, g, :], AF.Sigmoid)
            nc.vector.tensor_tensor(sg, sg, zf, OP.mult)
            nc.vector.tensor_tensor(
                y2[:, 0:4 * L], y2[:, 0:4 * L], sg[:, 0:4 * L], OP.mult)
            nc.vector.tensor_tensor(
                u2[:, 0:3 * L + 512],
                y_ps.rearrange("p s t -> p (s t)"),
                sg[:, 4 * L:7 * L + 512], OP.mult)
            nc.vector.tensor_tensor(
                u2[:, 3 * L + 512:4 * L], y2[:, 7 * L + 512:8 * L],
                sg[:, 7 * L + 512:8 * L], OP.mult)
        psumZ.release()
        psumY.release()

        # ========= phase C: fused (out_proj @ proj) matmul =========
        # gated y lives in y2[0:4L] (blocks 0-3) and u2[0:4L] (blocks 4-7)
        psumC = tc.alloc_tile_pool(name="psumC", bufs=6, space="PSUM")
        pT_ap = pT_d.ap().rearrange("(k p) t -> p k t", p=128)
        for m in range(8):
            w2m = w2_tiles[m]
            if m + 2 < 8:
                w2_tiles.append(w2_fetch(m + 2))
            stg = tail.tile([128, L], dt.float16, tag="stg", bufs=2)
            ps = psumC.tile([128, L], dt.float32, tag="mm2", bufs=3)
            for h in range(2):
                for k in range(8):
                    src = y2 if k < 4 else u2
                    off = (k if k < 4 else k - 4) * L
                    nc.tensor.matmul(
                        ps[:, h * 512:(h + 1) * 512],
                        w2m[:, k, :],
                        src[:, off + h * 512: off + (h + 1) * 512],
                        start=(k == 0),
                        stop=(k == 7),
                    )
            nc.scalar.copy(stg, ps)
            nc.sync.dma_start(pT_ap[:, m, :], stg)
        psumC.release()
        tail.release()
        phb.release()
        pha_late.release()
        dram.release()
        persist.release()
        const.release()

    nc.compile()
    return nc


def _conv_taps(conv_w):
    """(DI, 4) -> (128, 16, 4) fp32: [p, g, j] = conv_w[g*128+p, j]."""
    return np.ascontiguousarray(
        conv_w.reshape(NBLK_F, 128, 4).transpose(1, 0, 2), dtype=np.float32)


def _wxi_layout(w):
    """(D, nm*128) -> (nm, 128, 8, 128): [m, p, k, c] = w[k*128+p, m*128+c]
    so each m-block DMA reads contiguous 2KB per partition."""
    nm = w.shape[1] // 128
    return np.ascontiguousarray(
        w.reshape(8, 128, nm, 128).transpose(2, 1, 0, 3), dtype=F16)


def _a_imm(inputs):
    """If A = -exp(A_log) is identical across d and across all cores' slices,
    return the 16 per-state values to bake as immediates, else None."""
    al = np.float64(inputs["A_log"])
    A = (-np.exp(al)).astype(np.float32)       # (2, DI, NST)
    row = A[0, 0]
    if np.array_equal(A, np.broadcast_to(row, A.shape)):
        return tuple(float(v) for v in row)
    return None


def _prep_core_inputs(inputs, c, with_A):
    """Slice/permute/cast the full inputs for core c (all numpy, cheap)."""
    dr, b, half = c // 4, (c // 2) % 2, c % 2
    s0 = half * DH
    # d_inner permutation putting this core's half first
    perm = np.r_[DH:DI, 0:DH] if half == 1 else np.r_[0:DI]

    x = inputs["x"][b]
    if dr == 1:
        x = x[::-1]
    in_w = inputs["in_w"][dr]

    m = {
        "xT": np.ascontiguousarray(x.T, dtype=F16),
        "w_xi": _wxi_layout(in_w[:, :DI][:, perm]),
        "w_z": _wxi_layout(in_w[:, DI + s0:DI + s0 + DH]),
        "conv_w": _conv_taps(inputs["conv_w"][dr][perm]),
        "conv_b": np.ascontiguousarray(inputs["conv_b"][dr][perm], dtype=np.float32),
        "xp_w": _pad_xp(inputs["xp_w"][dr][perm]),
        "dt_w": np.ascontiguousarray(inputs["dt_w"][dr][:, s0:s0 + DH], dtype=F16),
        "dt_b": np.ascontiguousarray(inputs["dt_b"][dr][s0:s0 + DH], dtype=np.float32),
        "dskip": np.ascontiguousarray(inputs["D"][dr][s0:s0 + DH], dtype=np.float32),
        # fused (out_w_half @ proj_w_dir): one matmul on-device instead of two
        "w2": _wxi_layout(
            inputs["out_w"][dr][s0:s0 + DH].astype(np.float32)
            @ inputs["proj_w"][dr * D:(dr + 1) * D].astype(np.float32)),
        "ident": np.eye(128, dtype=F16),
        "sel": _sel_rows(),
        "dskip_diag": _dskip_diag(inputs["D"][dr][s0:s0 + DH]),
    }
    if with_A:
        A_full = -np.exp(np.float64(inputs["A_log"][dr])).astype(np.float32)
        m["A"] = np.ascontiguousarray(A_full[s0:s0 + DH], dtype=np.float32)
    return m


def _dskip_diag(dskip):
    """(DH,) -> (128, 4, 128) diag(D) for d-blocks 4..7 (PSUM y seed)."""
    out = np.zeros((128, 4, 128), F16)
    idx = np.arange(128)
    for i in range(4):
        out[idx, i, idx] = dskip[(4 + i) * 128:(5 + i) * 128].astype(F16)
    return out


def _sel_rows():
    """(NST, 2, 128) one-hot-row selectors: sel[k, j, :] = (k == j)."""
    out = np.zeros((NST, 2, 128), F16)
    out[0, 0, :] = 1.0
    out[1, 1, :] = 1.0
    return out


def _pad_xp(xp):
    """(DI, 96) -> (DI, 128) with C cols moved to 96 (PSUM partition-start
    alignment: compute engines can only read partitions starting at 0/32/64/96)."""
    out = np.zeros((DI, 128), F16)
    out[:, :RNK + NST] = xp[:, :RNK + NST]
    out[:, 96:96 + NST] = xp[:, RNK + NST:]
    return out


def _gather(inputs, results):
    out = np.zeros((B, L, D), np.float32)
    for c, res in enumerate(results):
        dr, b = c // 4, (c // 2) % 2
        p = res["pT"].astype(np.float32).T
        if dr == 1:
            p = p[::-1]
        out[b] += p
    out += inputs["proj_b"]
    return out


def kernel(**inputs):
    inputs = {k: np.asarray(v) for k, v in inputs.items()}
    a_imm = _a_imm(inputs)
    key = ("nc", a_imm)
    if key not in _CACHE:
        _CACHE[key] = _build_module(a_imm=a_imm)
    nc = _CACHE[key]
    in_maps = [_prep_core_inputs(inputs, c, with_A=a_imm is None) for c in range(8)]
    from concourse.bass_utils import run_bass_kernel_spmd
    res = run_bass_kernel_spmd(nc, in_maps, core_ids=list(range(8)))
    return _gather(inputs, res.results)


# revision 66
# speedup vs baseline: 1.0091x; 1.0072x over previous
"""BiMamba block on 8 Trainium2 NeuronCores via Bass/Tile.

Sharding (SPMD, one shared NEFF, no collectives):
  core c: dir = c//4 (0=fwd, 1=bwd), batch = (c//2)%2, half = c%2.
Each core runs the full mamba pipeline for one (dir, batch) pair on its
half of d_inner (scan channels are independent), computing the full-d_inner
xi/conv/x_proj path locally (dt/B/C need the full d_inner contraction).
The d_inner axis is permuted per core so its own half is always blocks 0..7,
keeping the program identical across cores. Each core emits a partial
output (d_model, L) = (y_half @ out_w_half) @ proj_w_dir, transposed;
the host sums the 8 partials, un-reverses the bwd direction, adds proj_b.

Layouts: everything on-chip is "transposed" (feature dim on partitions,
time on the free axis) so the causal conv is a free-dim shift, the scan
runs along the free axis (DVE tensor_tensor_scan), and every matmul uses
naturally-laid-out weights as the stationary lhsT operand.

v2 engine split (the DVE scan is the hard floor: 2 cyc/elem dependency-
bound, 17.2us per state, 16 states back-to-back = ~437us of the runtime):
  PE:  in_proj xi, x_proj, dt, selector/diag(D) broadcasts, z, identity
       y-accumulate, out_proj.
  DVE: causal conv (per-tap scalar_tensor_tensor), u/b/C mults, scan, y-add.
  ACT: xi PSUM evacuation, silu(+bias), softplus (Exp+Ln), dA exps, fused
       z silu-evacuation.
Per state n: dA = exp(A[:,n]*dt) (ACT, double-buffered one state ahead),
b = u*B_n (DVE, partition-broadcast B), h = scan(dA, b) with chain-reset
via GpSimd dA=0 memsets at block boundaries (a DVE memset would park the
in-order DVE queue on the exp), y += h*C_n.

y accumulation: blocks 0-3 DVE-add in SBUF fp16; blocks 4-6 + 7-lo via PE
identity matmuls into 7 PSUM banks; block 7-hi DVE-adds into y2's dead
second half. The 8th PSUM bank runs the z matmuls DURING the early scan
states. Pool-overlap deps are pool-granular (an alloc over a released zone
waits for the WHOLE old pool), so x^T + streamed w_z chunks live in their
own never-overlapped pool (pha_late) and the rest of phase A (pha) is
released right after u/seed, letting scan 0 start at ~129us.

B/C rows: states 0/1 are broadcast by PE one-hot-row selector matmuls
(the DMA broadcast has ~25us latency); later pairs are DMA-broadcast from
a DRAM staging buffer, prefetched one pair ahead on both HWDGE queues.
State 15 is split into two half scans and the output partial is fp16 so
the tail (gate + fused out_w@proj_w matmul, 2-bank PSUM groups, streamed
weights) finishes ~56us after the last scan starts.
"""

import numpy as np

B, L, D = 2, 1024, 1024
DI, DH, NST, RNK = 2048, 1024, 16, 64
NBLK = DH // 128          # 8 d-blocks per half
NBLK_F = DI // 128        # 16 d-blocks full
F16 = np.float16

_CACHE = {}


def _build_module(sim_compat=False, a_imm=None):
    """sim_compat=True replaces Silu (absent from CoreSim) with
    Sigmoid + multiply; the hardware build uses the Silu table directly."""
    import concourse.bass as bass
    import concourse.mybir as mybir
    from concourse import bacc
    from concourse.tile import TileContext

    dt = mybir.dt
    AF = mybir.ActivationFunctionType
    OP = mybir.AluOpType

    nc = bacc.Bacc("TRN2", target_bir_lowering=False, debug=False)

    # ---- DRAM I/O ----
    xT_d = nc.dram_tensor("xT", (D, L), dt.float16, kind="ExternalInput")
    w_xi_d = nc.dram_tensor("w_xi", (NBLK_F, 128, 8, 128), dt.float16, kind="ExternalInput")
    w_z_d = nc.dram_tensor("w_z", (8, 128, 8, 128), dt.float16, kind="ExternalInput")
    conv_w_d = nc.dram_tensor("conv_w", (128, NBLK_F, 4), dt.float32, kind="ExternalInput")
    conv_b_d = nc.dram_tensor("conv_b", (DI,), dt.float32, kind="ExternalInput")
    xp_w_d = nc.dram_tensor("xp_w", (DI, 128), dt.float16, kind="ExternalInput")
    dt_w_d = nc.dram_tensor("dt_w", (RNK, DH), dt.float16, kind="ExternalInput")
    dt_b_d = nc.dram_tensor("dt_b", (DH,), dt.float32, kind="ExternalInput")
    A_d = None
    if a_imm is None:
        A_d = nc.dram_tensor("A", (DH, NST), dt.float32, kind="ExternalInput")
    dskip_d = nc.dram_tensor("dskip", (DH,), dt.float32, kind="ExternalInput")
    w2_d = nc.dram_tensor("w2", (8, 128, 8, 128), dt.float16, kind="ExternalInput")
    ident_d = nc.dram_tensor("ident", (128, 128), dt.float16, kind="ExternalInput")
    # one-hot-row selectors: sel[k, j, :] = 1.0 iff k == j (j = state 0/1)
    sel_d = nc.dram_tensor("sel", (NST, 2, 128), dt.float16, kind="ExternalInput")
    # diag(D) per d-block 4..7, the PSUM seed weights for the y skip term
    dd_d = nc.dram_tensor("dskip_diag", (128, 4, 128), dt.float16, kind="ExternalInput")
    pT_d = nc.dram_tensor("pT", (D, L), dt.float16, kind="ExternalOutput")

    with TileContext(nc) as tc:
        psum = tc.alloc_tile_pool(name="psum", bufs=6, space="PSUM")
        const = tc.alloc_tile_pool(name="const", bufs=1)
        persist = tc.alloc_tile_pool(name="persist", bufs=1)
        dram = tc.alloc_tile_pool(name="dram", bufs=1, space="DRAM")
        # B/C rows staged in DRAM so they can be partition-broadcast by DMA
        bc_stage = dram.tile([2 * NST, L], dt.float16)

        # ---- constants read during the scan phase (everything phase-A-only
        # lives in pha_early so its space is reclaimed for the scan pools) ----
        A_sb = None
        if a_imm is None:
            A_sb = const.tile([128, NBLK, NST], dt.float32)
        dskip_sb = const.tile([128, NBLK], dt.float32)
        ident_sb = const.tile([128, 128], dt.float16)
        dd_sb = const.tile([128, 4, 128], dt.float16)

        # ---- persistent activations ----
        zT = persist.tile([128, NBLK, L], dt.float16)
        dtT = persist.tile([128, NBLK, L], dt.float16)
        u2 = persist.tile([128, NBLK * L], dt.float16)
        y2 = persist.tile([128, NBLK * L], dt.float16)
        u3 = u2.rearrange("p (g t) -> p g t", g=NBLK)
        y3 = y2.rearrange("p (g t) -> p g t", g=NBLK)
        # pair-0 B/C broadcasts (PE selector matmuls) alias y2 blocks 4-7,
        # which accumulate in PSUM and never touch their SBUF half; dA(0)
        # needs the persist pool (both are produced while the phase A pools
        # still hold all of SBUF, so they cannot live in phb)
        bc0 = y2[:, 4 * L:8 * L].rearrange("p (q t) -> p q t", q=4)
        dA0_p = persist.tile([128, NBLK * L], dt.float16)
        # block 7 upper-half skip term D*xc, staged until its y accumulator
        # region (the tail of bc0) frees up after state 1
        pad7skip = persist.tile([128, 512], dt.float16)

        # ================= phase A: in_proj, conv, x_proj, dt =================
        # Pool-overlap dependencies are pool-granular: a pool allocated over
        # a released zone waits for the WHOLE old pool to drain. The z
        # matmuls keep reading x^T until deep into the scan phase, so x^T
        # (plus the streamed w_z chunks) lives in its own pool, kept alive
        # until the end; everything else phase-A is in pha_early, released
        # right after dbc/u so the scan pools start immediately.
        pha_late = tc.alloc_tile_pool(name="pha_late", bufs=1)
        xT_sb = pha_late.tile([128, 8, L], dt.float16)
        xT_ap = xT_d.ap().rearrange("(k p) t -> p k t", p=128)
        nc.sync.dma_start(xT_sb[:, :, 0:512], xT_ap[:, :, 0:512])

        pha = tc.alloc_tile_pool(name="pha", bufs=1)
        conv_b_sb = pha.tile([128, NBLK_F], dt.float32)
        nc.sync.dma_start(conv_b_sb, conv_b_d.ap().rearrange("(g p) -> p g", p=128))
        conv_w_sb = pha.tile([128, NBLK_F, 4], dt.float32)
        nc.sync.dma_start(conv_w_sb, conv_w_d.ap())
        dt_w_sb = pha.tile([RNK, DH], dt.float16)
        dt_b_sb = pha.tile([128, NBLK], dt.float32)
        sel_sb = pha.tile([NST, 2, 128], dt.float16)
        BT = pha.tile([NST, L], dt.float16)
        CT = pha.tile([NST, L], dt.float16)
        dtrT = pha.tile([RNK, L], dt.float16)
        xp_w_sb = pha.tile([128, NBLK_F, 128], dt.float16)
        xc = pha.tile([128, NBLK_F, L], dt.float16)

        def wxi_fetch(m):
            wxi_t = pha.tile([128, 8, 128], dt.float16, tag="wxi", bufs=3)
            nc.sync.dma_start(wxi_t, w_xi_d.ap()[m])
            return wxi_t

        # queue order: xT half 0, first two weight blocks, then the rest of
        # the bulk (block 0's matmuls only need xT[:, :, 0:512] + wxi_0)
        wxi_tiles = [wxi_fetch(0), wxi_fetch(1)]
        nc.sync.dma_start(xT_sb[:, :, 512:L], xT_ap[:, :, 512:L])
        nc.sync.dma_start(xp_w_sb, xp_w_d.ap().rearrange("(g p) j -> p g j", p=128))

        # x_proj accumulates incrementally inside the conv loop (the psum
        # group stays open across it), so dbc completes right after the last
        # conv block instead of via serial matmuls later.
        ps96_h0 = psum.tile([128, 512], dt.float32, tag="xp", bufs=2)
        ps96_h1 = psum.tile([128, 512], dt.float32, tag="xp", bufs=2)
        ps96 = [ps96_h0, ps96_h1]

        # xi blocks stream through PE; the 4-tap causal conv runs on DVE
        # (idle in phase A) as scalar_tensor_tensor chains, with ACT doing
        # the PSUM evacuation and the silu+bias.
        for m in range(NBLK_F):
            wxi_m = wxi_tiles[m]
            if m + 2 < NBLK_F:
                wxi_tiles.append(wxi_fetch(m + 2))
            xi_pad = pha.tile([128, 1028], dt.float16, tag="xi_pad", bufs=3)
            nc.vector.memset(xi_pad[:, 0:4], 0.0)
            for h in range(2):
                ps = psum.tile([128, 512], dt.float32, tag="mm")
                for k in range(8):
                    nc.tensor.matmul(
                        ps,
                        wxi_m[:, k, :],
                        xT_sb[:, k, h * 512:(h + 1) * 512],
                        start=(k == 0),
                        stop=(k == 7),
                    )
                nc.scalar.copy(xi_pad[:, 4 + h * 512: 4 + (h + 1) * 512], ps)
            # 4-tap conv as tensor_scalar + pairwise TT adds (all ops have
            # 2x/4x DVE uops; scalar_tensor_tensor only has 1x). Costs one
            # extra fp16 rounding vs an STT chain: ~2e-4 of relative error.
            cacc = pha.tile([128, L], dt.float16, tag="cacc", bufs=3)
            ct0 = pha.tile([128, L], dt.float16, tag="ct0", bufs=3)
            ct1 = pha.tile([128, L], dt.float16, tag="ct1", bufs=3)
            nc.vector.tensor_scalar(
                ct0, xi_pad[:, 1:1 + L], conv_w_sb[:, m, 0:1], None, OP.mult)
            nc.vector.tensor_scalar(
                ct1, xi_pad[:, 2:2 + L], conv_w_sb[:, m, 1:2], None, OP.mult)
            nc.vector.tensor_tensor(ct0, ct0, ct1, OP.add)
            nc.vector.tensor_scalar(
                ct1, xi_pad[:, 3:3 + L], conv_w_sb[:, m, 2:3], None, OP.mult)
            nc.vector.tensor_scalar(
                cacc, xi_pad[:, 4:4 + L], conv_w_sb[:, m, 3:4], None, OP.mult)
            nc.vector.tensor_tensor(ct1, ct1, cacc, OP.add)
            nc.vector.tensor_tensor(cacc, ct0, ct1, OP.add)
            if sim_compat:
                sg = pha.tile([128, L], dt.float16, tag="conv_sg", bufs=3)
                nc.scalar.activation(sg, cacc, AF.Sigmoid, bias=conv_b_sb[:, m:m + 1])
                nc.vector.scalar_tensor_tensor(
                    xc[:, m, :], cacc, conv_b_sb[:, m:m + 1], sg, OP.add, OP.mult
                )
            else:
                nc.scalar.activation(
                    xc[:, m, :], cacc, AF.Silu, bias=conv_b_sb[:, m:m + 1]
                )
            for h in range(2):
                nc.tensor.matmul(
                    ps96[h],
                    xp_w_sb[:, m, :],
                    xc[:, m, h * 512:(h + 1) * 512],
                    start=(m == 0),
                    stop=(m == NBLK_F - 1),
                    skip_group_check=True,
                )
            # deferred bulk DMAs, issued mid-stream so they neither delay the
            # first xi blocks nor arrive late for their consumers
            if m == 10:
                nc.sync.dma_start(dt_w_sb, dt_w_d.ap())
                nc.sync.dma_start(dt_b_sb, dt_b_d.ap().rearrange("(g p) -> p g", p=128))
                if a_imm is None:
                    nc.sync.dma_start(A_sb, A_d.ap().rearrange("(g p) n -> p g n", p=128))
                nc.sync.dma_start(dskip_sb, dskip_d.ap().rearrange("(g p) -> p g", p=128))
                nc.sync.dma_start(ident_sb, ident_d.ap())
                nc.sync.dma_start(sel_sb, sel_d.ap())
                nc.sync.dma_start(dd_sb, dd_d.ap())

        # ---- dbc -> dt/B/C; B/C round-trip through DRAM for broadcast ----
        for h in range(2):
            nc.scalar.copy(dtrT[:, h * 512:(h + 1) * 512], ps96[h][0:RNK, :])
            nc.vector.tensor_copy(BT[:, h * 512:(h + 1) * 512], ps96[h][RNK:RNK + NST, :])
            nc.vector.tensor_copy(CT[:, h * 512:(h + 1) * 512], ps96[h][96:96 + NST, :])
        nc.sync.dma_start(bc_stage[0:NST, :], BT)
        nc.sync.dma_start(bc_stage[NST:2 * NST, :], CT)

        # y skip term D * xc: blocks 0-3 initialize y2 (DVE path); blocks 4-6
        # and 7-lo are seeded into PSUM later by diag(D) matmuls straight
        # from xc; block 7-hi is staged (its SBUF accumulator region is
        # still occupied by the bc0 broadcasts until state 1).
        for g in range(4):
            nc.vector.tensor_scalar(
                y3[:, g, :], xc[:, g, :], dskip_sb[:, g:g + 1], None, OP.mult
            )
        nc.vector.tensor_scalar(
            pad7skip, xc[:, 7, 512:L], dskip_sb[:, 7:8], None, OP.mult)

        # dt^T = softplus(dt_w^T @ dt_raw^T + dt_b), as Ln(Exp(v)+1)
        # (no Softplus table on this build; v <= ~-1 here so Exp can't
        # overflow). All 16 Exps run before the Lns: the ACT table chooser
        # reloads on every Exp<->Ln switch; batching needs 2 loads.
        # ev8 reuses xc blocks 8-15, dead once their x_proj matmuls ran.
        ev8 = xc[:, NBLK:NBLK_F, :]
        for m in range(NBLK):
            for h in range(2):
                ps = psum.tile([128, 512], dt.float32, tag="mm")
                nc.tensor.matmul(
                    ps,
                    dt_w_sb[:, m * 128:(m + 1) * 128],
                    dtrT[:, h * 512:(h + 1) * 512],
                    start=True,
                    stop=True,
                )
                nc.scalar.activation(
                    ev8[:, m, h * 512:(h + 1) * 512], ps, AF.Exp,
                    bias=dt_b_sb[:, m:m + 1],
                )
        dtT2 = dtT.rearrange("p g t -> p (g t)")
        ev8f = ev8.rearrange("p g t -> p (g t)")
        nc.scalar.activation(dtT2[:, 0:4 * L], ev8f[:, 0:4 * L], AF.Ln, bias=1.0)
        nc.scalar.activation(dtT2[:, 4 * L:8 * L], ev8f[:, 4 * L:8 * L], AF.Ln, bias=1.0)

        # u = dt * xc_half as ONE flat TT
        nc.vector.tensor_tensor(
            u2, dtT2, xc[:, 0:NBLK, :].rearrange("p g t -> p (g t)"), OP.mult)

        # pair-0 (and pair-1's first row is state 1) B/C broadcasts via PE:
        # out[p,t] = sum_k sel[k,n,p]*BT[k,t] = BT[n,t] for every partition.
        # The DMA-broadcast path used for later pairs has ~25us latency; this
        # costs 8 tiny matmuls + ACT evacs while PSUM is still free.
        for j, (row, qi) in enumerate([(BT, 0), (CT, 2), (BT, 1), (CT, 3)]):
            nst = j // 2  # B0, C0 are state 0; B1, C1 are state 1
            for h in range(2):
                ps = psum.tile([128, 512], dt.float32, tag="mm")
                nc.tensor.matmul(
                    ps, sel_sb[:, nst, :], row[:, h * 512:(h + 1) * 512],
                    start=True, stop=True,
                )
                nc.scalar.copy(bc0[:, qi, h * 512:(h + 1) * 512], ps)
            if j == 3:
                # dA(0) after all four B/C evacs: the evacs also gate
                # psum.release -> seed -> pha drain -> b0, so putting the
                # 7us exp between them would push scan 0 out by that much
                if a_imm is not None:
                    nc.scalar.activation(dA0_p, dtT2, AF.Exp, scale=float(a_imm[0]))
                else:
                    dA0_3 = dA0_p.rearrange("p (g t) -> p g t", g=NBLK)
                    for g in range(NBLK):
                        nc.scalar.activation(
                            dA0_3[:, g, :], dtT[:, g, :], AF.Exp,
                            scale=A_sb[:, g, 0:1])

        psum.release()
        # y blocks 4-6 and 7-lo accumulate in 7 PSUM banks (block 7-hi goes
        # through SBUF on DVE), leaving one bank for the z matmuls, which
        # then run concurrently with the first scan states instead of
        # blocking the whole pipeline warm-up. Seed = diag(D) @ xc.
        psumY = tc.alloc_tile_pool(name="psumY", bufs=1, space="PSUM")
        y_ps = psumY.tile([128, 7, 512], dt.float32)
        for s in range(7):
            nc.tensor.matmul(
                y_ps[:, s], dd_sb[:, s // 2, :],
                xc[:, 4 + s // 2, (s % 2) * 512:(s % 2) * 512 + 512],
                start=True, stop=False, skip_group_check=True,
            )
        psumZ = tc.alloc_tile_pool(name="psumZ", bufs=1, space="PSUM")
        pha.release()

        # ---- scan-phase + tail pools (pha space reused) ----
        phb = tc.alloc_tile_pool(name="phb", bufs=2)
        tail = tc.alloc_tile_pool(name="tail", bufs=1)

        def w2_fetch(m):
            w2m = tail.tile([128, 8, 128], dt.float16, tag="w2m", bufs=2)
            nc.sync.dma_start(w2m, w2_d.ap()[m])
            return w2m

        def bc_fetch(pair):
            # B on the SP queue, C on the ACT queue: each broadcast reads its
            # 2 source rows 128x, so one fetch is ~25us per queue; the split
            # halves it and the pair is prefetched a full state ahead anyway
            brt = tail.tile([128, 2, L], dt.float16, tag="brep", bufs=2)
            nc.sync.dma_start(
                brt, bc_stage[2 * pair:2 * pair + 2, :]
                .unsqueeze(0).broadcast_to((128, 2, L)))
            crt = tail.tile([128, 2, L], dt.float16, tag="crep", bufs=2)
            nc.scalar.dma_start(
                crt, bc_stage[NST + 2 * pair:NST + 2 * pair + 2, :]
                .unsqueeze(0).broadcast_to((128, 2, L)))
            return brt, crt

        b_t = phb.tile([128, NBLK * L], dt.float16, tag="b")
        # dA double-buffer: even states reuse the persist-pool dA0_p
        dA_b = phb.tile([128, NBLK * L], dt.float16, tag="dA", bufs=1)

        def dA_tile(n):
            return dA0_p if n % 2 == 0 else dA_b

        def dA_exp(n):
            dA_t = dA_tile(n)
            dA3 = dA_t.rearrange("p (g t) -> p g t", g=NBLK)
            if a_imm is not None:
                nc.scalar.activation(dA_t, dtT2, AF.Exp, scale=float(a_imm[n]))
            else:
                for g in range(NBLK):
                    nc.scalar.activation(
                        dA3[:, g, :], dtT[:, g, :], AF.Exp, scale=A_sb[:, g, n:n + 1]
                    )
            # reset the recurrence at each chained d-block boundary
            # (GpSimd: a DVE memset would park the in-order DVE queue on
            # this dA's exp, stalling the next scan behind it)
            nc.gpsimd.memset(dA_t[:, 0:NBLK * L:L], 0.0)

        nc.vector.tensor_tensor(
            b_t.rearrange("p (g t) -> p g t", g=NBLK), u3,
            bc0[:, 0].unsqueeze(1).broadcast_to((128, NBLK, L)), OP.mult)
        # dA0's boundary memset, after b0 so the DVE queue doesn't park on
        # the dA0 exp; dA1 next so ACT runs it ahead of the z evacuations
        nc.gpsimd.memset(dA0_p[:, 0:NBLK * L:L], 0.0)
        dA_exp(1)

        # z = x @ w_z (z^T = w_z^T @ x^T): PE + its single PSUM bank churn
        # through these in the background of the first scan states; the
        # evacuation is fused with the gate's silu in one ACT op per tile.
        for m in range(NBLK):
            wzm = pha_late.tile([128, 8, 128], dt.float16, tag="wzm", bufs=2)
            nc.sync.dma_start(wzm, w_z_d.ap()[m])
            for h in range(2):
                ps = psumZ.tile([128, 512], dt.float32, tag="zmm")
                for k in range(8):
                    nc.tensor.matmul(
                        ps,
                        wzm[:, k, :],
                        xT_sb[:, k, h * 512:(h + 1) * 512],
                        start=(k == 0),
                        stop=(k == 7),
                    )
                if sim_compat:
                    nc.scalar.copy(zT[:, m, h * 512:(h + 1) * 512], ps)
                else:
                    nc.scalar.activation(zT[:, m, h * 512:(h + 1) * 512], ps, AF.Silu)

        bc_next = bc_fetch(1)
        # first out_proj weight chunks, streamed during the scan phase
        w2_tiles = [w2_fetch(0), w2_fetch(1)]

        # ================= phase B: selective scan over n =================
        zf = zT.rearrange("p g t -> p (g t)")
        bc_cur = None
        for n in range(NST):
            if n >= 2 and n % 2 == 0:
                bc_cur = bc_next
                if n + 2 < NST:
                    bc_next = bc_fetch(n // 2 + 1)
            if n < 2:
                B_rep, C_rep = bc0[:, n], bc0[:, 2 + n]
            else:
                B_rep, C_rep = bc_cur[0][:, n % 2], bc_cur[1][:, n % 2]

            dA_t = dA_tile(n)
            h = phb.tile([128, NBLK * L], dt.float16, tag="h")
            h3 = h.rearrange("p (g t) -> p g t", g=NBLK)
            if n < NST - 1:
                nc.vector.tensor_tensor_scan(h, dA_t, b_t, 0.0, OP.mult, OP.add)
                # next state's b while this scan's consumers wait
                nb = phb.tile([128, NBLK * L], dt.float16, tag="b")
                if n == 0:
                    nB_rep = bc0[:, 1]
                else:
                    nB_rep = (bc_next if (n + 1) % 2 == 0 else bc_cur)[0][:, (n + 1) % 2]
                nc.vector.tensor_tensor(
                    nb.rearrange("p (g t) -> p g t", g=NBLK), u3,
                    nB_rep.unsqueeze(1).broadcast_to((128, NBLK, L)), OP.mult)
                nc.vector.tensor_tensor(
                    h3, h3, C_rep.unsqueeze(1).broadcast_to((128, NBLK, L)), OP.mult
                )
                nc.vector.tensor_tensor(
                    y2[:, 0:4 * L], y2[:, 0:4 * L], h[:, 0:4 * L], OP.add)
                # block 7-hi rides in SBUF (no PSUM bank left); its region is
                # bc0's C1 row, free only after state 1 consumed it
                if n == 1:
                    nc.vector.tensor_tensor(
                        y2[:, 7 * L + 512:8 * L], h_prev[:, 7 * L + 512:8 * L],
                        h[:, 7 * L + 512:8 * L], OP.add)
                elif n >= 2:
                    nc.vector.tensor_tensor(
                        y2[:, 7 * L + 512:8 * L], y2[:, 7 * L + 512:8 * L],
                        h[:, 7 * L + 512:8 * L], OP.add)
                for s in range(7):
                    nc.tensor.matmul(
                        y_ps[:, s], ident_sb,
                        h[:, 4 * L + s * 512: 4 * L + (s + 1) * 512],
                        start=False, stop=False, skip_group_check=True,
                    )
                # dA(n+2) prefetch, issued last: its DVE memset must queue
                # behind this iteration's b/C/y-add (the exp itself waits for
                # scan n to free the buffer, so an early memset would stall
                # the whole DVE queue on that exp)
                if n + 2 < NST:
                    dA_exp(n + 2)
                b_t = nb
                h_prev = h
            else:
                # last state: split the scan so the tail starts on blocks 0-3
                # while blocks 4-7 still stream through PE/PSUM
                nc.vector.tensor_tensor_scan(
                    h[:, 0:4 * L], dA_t[:, 0:4 * L], b_t[:, 0:4 * L],
                    0.0, OP.mult, OP.add)
                nc.vector.tensor_tensor_scan(
                    h[:, 4 * L:8 * L], dA_t[:, 4 * L:8 * L], b_t[:, 4 * L:8 * L],
                    0.0, OP.mult, OP.add)
                nc.vector.tensor_tensor(
                    h3[:, 0:4], h3[:, 0:4],
                    C_rep.unsqueeze(1).broadcast_to((128, 4, L)), OP.mult)
                nc.vector.tensor_tensor(
                    y2[:, 0:4 * L], y2[:, 0:4 * L], h[:, 0:4 * L], OP.add)
                if not sim_compat:
                    # gate blocks 0-3 in place: y2 *= silu(z)
                    nc.vector.tensor_tensor(
                        y2[:, 0:4 * L], y2[:, 0:4 * L], zf[:, 0:4 * L], OP.mult)
                nc.vector.tensor_tensor(
                    h3[:, 4:8], h3[:, 4:8],
                    C_rep.unsqueeze(1).broadcast_to((128, 4, L)), OP.mult)
                nc.vector.tensor_tensor(
                    y2[:, 7 * L + 512:8 * L], y2[:, 7 * L + 512:8 * L],
                    h[:, 7 * L + 512:8 * L], OP.add)
                for s in range(7):
                    nc.tensor.matmul(
                        y_ps[:, s], ident_sb,
                        h[:, 4 * L + s * 512: 4 * L + (s + 1) * 512],
                        start=False, stop=True, skip_group_check=True,
                    )
                nc.vector.tensor_tensor(
                    y2[:, 7 * L + 512:8 * L], y2[:, 7 * L + 512:8 * L],
                    pad7skip, OP.add)
                if not sim_compat:
                    # gate blocks 4-7lo straight from PSUM into dead u2
                    # space, block 7-hi from its SBUF accumulator
                    nc.vector.tensor_tensor(
                        u2[:, 0:3 * L + 512],
                        y_ps.rearrange("p s t -> p (s t)"),
                        zf[:, 4 * L:7 * L + 512], OP.mult)
                    nc.vector.tensor_tensor(
                        u2[:, 3 * L + 512:4 * L], y2[:, 7 * L + 512:8 * L],
                        zf[:, 7 * L + 512:8 * L], OP.mult)

        if sim_compat:
            sg = phb.tile([128, NBLK * L], dt.float16, tag="h")
            sg3 = sg.rearrange("p (g t) -> p g t", g=NBLK)
            for g in range(NBLK):
                nc.scalar.activation(sg3[:, g, :], zT[:, g, :], AF.Sigmoid)
            nc.vector.tensor_tensor(sg, sg, zf, OP.mult)
            nc.vector.tensor_tensor(
                y2[:, 0:4 * L], y2[:, 0:4 * L], sg[:, 0:4 * L], OP.mult)
            nc.vector.tensor_tensor(
                u2[:, 0:3 * L + 512],
                y_ps.rearrange("p s t -> p (s t)"),
                sg[:, 4 * L:7 * L + 512], OP.mult)
            nc.vector.tensor_tensor(
                u2[:, 3 * L + 512:4 * L], y2[:, 7 * L + 512:8 * L],
                sg[:, 7 * L + 512:8 * L], OP.mult)
        psumZ.release()
        psumY.release()

        # ========= phase C: fused (out_proj @ proj) matmul =========
        # gated y lives in y2[0:4L] (blocks 0-3) and u2[0:4L] (blocks 4-7)
        psumC = tc.alloc_tile_pool(name="psumC", bufs=6, space="PSUM")
        pT_ap = pT_d.ap().rearrange("(k p) t -> p k t", p=128)
        for m in range(8):
            w2m = w2_tiles[m]
            if m + 2 < 8:
                w2_tiles.append(w2_fetch(m + 2))
            stg = tail.tile([128, L], dt.float16, tag="stg", bufs=2)
            ps = psumC.tile([128, L], dt.float32, tag="mm2", bufs=3)
            for h in range(2):
                for k in range(8):
                    src = y2 if k < 4 else u2
                    off = (k if k < 4 else k - 4) * L
                    nc.tensor.matmul(
                        ps[:, h * 512:(h + 1) * 512],
                        w2m[:, k, :],
                        src[:, off + h * 512: off + (h + 1) * 512],
                        start=(k == 0),
                        stop=(k == 7),
                    )
            nc.scalar.copy(stg, ps)
            nc.sync.dma_start(pT_ap[:, m, :], stg)
        psumC.release()
        tail.release()
        phb.release()
        pha_late.release()
        dram.release()
        persist.release()
        const.release()

    nc.compile()
    return nc


def _conv_taps(conv_w):
    """(DI, 4) -> (128, 16, 4) fp32: [p, g, j] = conv_w[g*128+p, j]."""
    return np.ascontiguousarray(
        conv_w.reshape(NBLK_F, 128, 4).transpose(1, 0, 2), dtype=np.float32)


def _wxi_layout(w):
    """(D, nm*128) -> (nm, 128, 8, 128): [m, p, k, c] = w[k*128+p, m*128+c]
    so each m-block DMA reads contiguous 2KB per partition."""
    nm = w.shape[1] // 128
    return np.ascontiguousarray(
        w.reshape(8, 128, nm, 128).transpose(2, 1, 0, 3), dtype=F16)


def _a_imm(inputs):
    """If A = -exp(A_log) is identical across d and across all cores' slices,
    return the 16 per-state values to bake as immediates, else None."""
    al = np.float64(inputs["A_log"])
    A = (-np.exp(al)).astype(np.float32)       # (2, DI, NST)
    row = A[0, 0]
    if np.array_equal(A, np.broadcast_to(row, A.shape)):
        return tuple(float(v) for v in row)
    return None


def _prep_core_inputs(inputs, c, with_A):
    """Slice/permute/cast the full inputs for core c (all numpy, cheap)."""
    dr, b, half = c // 4, (c // 2) % 2, c % 2
    s0 = half * DH
    # d_inner permutation putting this core's half first
    perm = np.r_[DH:DI, 0:DH] if half == 1 else np.r_[0:DI]

    x = inputs["x"][b]
    if dr == 1:
        x = x[::-1]
    in_w = inputs["in_w"][dr]

    m = {
        "xT": np.ascontiguousarray(x.T, dtype=F16),
        "w_xi": _wxi_layout(in_w[:, :DI][:, perm]),
        "w_z": _wxi_layout(in_w[:, DI + s0:DI + s0 + DH]),
        "conv_w": _conv_taps(inputs["conv_w"][dr][perm]),
        "conv_b": np.ascontiguousarray(inputs["conv_b"][dr][perm], dtype=np.float32),
        "xp_w": _pad_xp(inputs["xp_w"][dr][perm]),
        "dt_w": np.ascontiguousarray(inputs["dt_w"][dr][:, s0:s0 + DH], dtype=F16),
        "dt_b": np.ascontiguousarray(inputs["dt_b"][dr][s0:s0 + DH], dtype=np.float32),
        "dskip": np.ascontiguousarray(inputs["D"][dr][s0:s0 + DH], dtype=np.float32),
        # fused (out_w_half @ proj_w_dir): one matmul on-device instead of two
        "w2": _wxi_layout(
            inputs["out_w"][dr][s0:s0 + DH].astype(np.float32)
            @ inputs["proj_w"][dr * D:(dr + 1) * D].astype(np.float32)),
        "ident": np.eye(128, dtype=F16),
        "sel": _sel_rows(),
        "dskip_diag": _dskip_diag(inputs["D"][dr][s0:s0 + DH]),
    }
    if with_A:
        A_full = -np.exp(np.float64(inputs["A_log"][dr])).astype(np.float32)
        m["A"] = np.ascontiguousarray(A_full[s0:s0 + DH], dtype=np.float32)
    return m


def _dskip_diag(dskip):
    """(DH,) -> (128, 4, 128) diag(D) for d-blocks 4..7 (PSUM y seed)."""
    out = np.zeros((128, 4, 128), F16)
    idx = np.arange(128)
    for i in range(4):
        out[idx, i, idx] = dskip[(4 + i) * 128:(5 + i) * 128].astype(F16)
    return out


def _sel_rows():
    """(NST, 2, 128) one-hot-row selectors: sel[k, j, :] = (k == j)."""
    out = np.zeros((NST, 2, 128), F16)
    out[0, 0, :] = 1.0
    out[1, 1, :] = 1.0
    return out


def _pad_xp(xp):
    """(DI, 96) -> (DI, 128) with C cols moved to 96 (PSUM partition-start
    alignment: compute engines can only read partitions starting at 0/32/64/96)."""
    out = np.zeros((DI, 128), F16)
    out[:, :RNK + NST] = xp[:, :RNK + NST]
    out[:, 96:96 + NST] = xp[:, RNK + NST:]
    return out


def _gather(inputs, results):
    out = np.zeros((B, L, D), np.float32)
    for c, res in enumerate(results):
        dr, b = c // 4, (c // 2) % 2
        p = res["pT"].astype(np.float32).T
        if dr == 1:
            p = p[::-1]
        out[b] += p
    out += inputs["proj_b"]
    return out


def kernel(**inputs):
    inputs = {k: np.asarray(v) for k, v in inputs.items()}
    a_imm = _a_imm(inputs)
    key = ("nc", a_imm)
    if key not in _CACHE:
        _CACHE[key] = _build_module(a_imm=a_imm)
    nc = _CACHE[key]
    in_maps = [_prep_core_inputs(inputs, c, with_A=a_imm is None) for c in range(8)]
    from concourse.bass_utils import run_bass_kernel_spmd
    res = run_bass_kernel_spmd(nc, in_maps, core_ids=list(range(8)))
    return _gather(inputs, res.results)
